# revision 1
# baseline (speedup 1.0000x reference)
"""Tensor-parallel GQA attention prefill (B=1, T=2048, D=4096, 32 q-heads /
8 kv-heads) for 8 Trainium2 NeuronCores.

Sharding: head-parallel.  Core c owns q-heads [4c, 4c+4) and kv-head c.
  phase 1: Q/K/V projections in transposed layout (head-dim on partitions),
           RoPE applied via a rotation-matmul + two table multiplies.
  phase 2: per-head attention with scores held transposed (tk on
           partitions); softmax denominators come from a ones-matmul;
           fully-masked tiles are skipped (host inspects the mask tensor).
  phase 2.5: per-tq-chunk AllGather of attention slices, pipelined with
           the remaining attention chunks.
  phase 3: output projection column-shard per core, consuming each
           gathered chunk as it lands.

Matmul operands are bf16 (fp32 accumulation in PSUM); measured end-to-end
error vs the fp32 reference is ~4e-3 relative.

NOTE: faithful to the reference "bug" -- the q projection uses wo_w/wo_b.
"""

import numpy as np
import ml_dtypes

import bass_rust
import concourse.bass as bass
import concourse.mybir as mybir
import concourse.tile as tile
from concourse.bass_utils import run_bass_kernel_spmd
from concourse.masks import make_identity

# problem constants (self-contained; do not read spec.json)
DIM = 4096
NH = 32
NKV = 8
HD = 128
T = 2048
NCORE = 8
HPC = NH // NCORE      # 4 q heads per core
JPC = HPC * HD         # 512 output columns per core
P = 128
NT = T // 512          # 4 free-dim chunks of 512
NKC = DIM // P         # 32 contraction chunks in the projections
TKC = T // P           # 16 tk chunks in attention
SCALE = 1.0 / float(np.sqrt(HD))

F32 = mybir.dt.float32
BF16 = mybir.dt.bfloat16

# mask tile classification
MSK_SKIP, MSK_ZERO, MSK_ADD = 0, 1, 2


def legalize_waits(nc, max_waits=1):
    """Hoist excess on_wait conditions onto preceding nop instructions.

    This walrus build rejects instructions carrying more than a couple of
    sync-wait commands; engines execute their queue in order, so a nop that
    waits immediately before the real instruction is equivalent.
    """
    n_new = 0
    for f in nc.m.functions:
        for bb in f.blocks:
            insts = bb.instructions
            new = []
            for ins in list(insts):
                si = ins.sync_info
                waits = list(si.on_wait) if si is not None and si.on_wait else []
                if len(waits) > max_waits:
                    hoist = waits[:-max_waits]
                    keep = waits[-max_waits:]
                    for j in range(0, len(hoist), max_waits):
                        chunk = hoist[j:j + max_waits]
                        nop = mybir.InstNoOp(
                            name=f"{ins.name}_hw{j}",
                            engine=ins.engine,
                            sync_info=bass_rust.SyncInfo(
                                on_wait=chunk, on_update=[]),
                        )
                        new.append(nop)
                        n_new += 1
                    ins.sync_info = bass_rust.SyncInfo(
                        on_wait=keep,
                        on_update=list(si.on_update) if si.on_update else [])
                new.append(ins)
            insts.clear()
            insts.extend(new)
    return n_new


def _classify_mask(mask):
    """Per (tk-chunk, tq-chunk-of-512) classification of the additive mask.

    Returns (klass, col0) where col0[k, c] is the first tq column (multiple
    of 128) of the chunk that is not fully masked -- matmuls/exp for the
    columns before it are skipped (their softmax weights are exactly 0).
    """
    klass = np.empty((TKC, NT), dtype=np.int32)
    col0 = np.zeros((TKC, NT), dtype=np.int32)
    for k in range(TKC):
        for c in range(NT):
            blk = mask[c * 512:(c + 1) * 512, k * P:(k + 1) * P]
            mx = float(blk.max())
            mn = float(blk.min())
            if mx < -80.0:
                klass[k, c] = MSK_SKIP
                continue
            if mx == 0.0 and mn == 0.0:
                klass[k, c] = MSK_ZERO
            else:
                klass[k, c] = MSK_ADD
            # leading fully-masked tq columns, rounded down to 128
            colmax = blk.max(axis=1)          # per-tq-row max over this tile
            nz = np.nonzero(colmax >= -80.0)[0]
            first = int(nz[0]) if len(nz) else 0
            first = (first // P) * P
            # only safe to skip if every column before `first` is fully masked
            if first > 0 and float(blk[:first].max()) < -80.0:
                col0[k, c] = first
    # never allow a fully-empty (all-skip) tq chunk; keep one tile live
    for c in range(NT):
        if all(klass[k, c] == MSK_SKIP for k in range(TKC)):
            klass[min(c * 4, TKC - 1), c] = MSK_ADD
    return klass, col0


def _build_module(klass, col0, phases=(1, 2, 25, 3)):
    nc = bass.Bass()

    # inputs are pre-reblocked on the host so every DMA is contiguous
    xTb = nc.declare_dram_parameter("xTb", [NT, DIM, 512], BF16, isOutput=False)
    woT = nc.declare_dram_parameter("woT", [DIM, JPC], BF16, isOutput=False)
    wkT = nc.declare_dram_parameter("wkT", [DIM, HD], BF16, isOutput=False)
    wvT = nc.declare_dram_parameter("wvT", [DIM, HD], BF16, isOutput=False)
    qb = nc.declare_dram_parameter("qb", [P, HPC], F32, isOutput=False)
    kb = nc.declare_dram_parameter("kb", [P, 1], F32, isOutput=False)
    vb = nc.declare_dram_parameter("vb", [P, 1], F32, isOutput=False)
    maskTb = nc.declare_dram_parameter("maskTb", [NT, T, 512], BF16,
                                       isOutput=False)
    cost = nc.declare_dram_parameter("cost", [P, T], F32, isOutput=False)
    sint = nc.declare_dram_parameter("sint", [P, T], F32, isOutput=False)
    rT = nc.declare_dram_parameter("rT", [P, P], BF16, isOutput=False)
    outTb = nc.declare_dram_parameter("outTb", [NT, JPC, 512], F32,
                                      isOutput=True)

    ag_in = nc.dram_tensor("ag_in", [NT, JPC, 512], BF16)
    ag_out = nc.dram_tensor("ag_out", [NT, NCORE * JPC, 512], BF16,
                            addr_space="Shared")

    with tile.TileContext(nc) as tc:
        with (
            tc.tile_pool(name="wpool", bufs=1) as wpool,
            tc.tile_pool(name="const", bufs=1) as constp,
            tc.tile_pool(name="qkv", bufs=1) as qkvp,
            tc.tile_pool(name="xs", bufs=8) as xsp,
            tc.tile_pool(name="stage", bufs=4) as stagep,
            tc.tile_pool(name="att", bufs=4) as attp,
            tc.tile_pool(name="acc", bufs=4, space="PSUM") as accp,
            tc.tile_pool(name="satt", bufs=2, space="PSUM") as sattp,
            tc.tile_pool(name="attden", bufs=2, space="PSUM") as adp,
        ):
            # ---- resident weights / tables -------------------------------
            # wo split with tiny leading pieces so the first matmuls start
            # as soon as possible
            wo_s = wpool.tile([P, NKC, JPC], BF16)
            for klo, khi in ((0, 1), (1, 2), (2, 4), (4, 8), (8, 16), (16, 32)):
                nc.sync.dma_start(
                    out=wo_s[:, klo:khi, :],
                    in_=woT[klo * P:khi * P, :].rearrange("(k p) j -> p k j",
                                                          p=P))
            wk_s = wpool.tile([P, NKC, HD], BF16)
            nc.sync.dma_start(out=wk_s, in_=wkT[:, :].rearrange(
                "(k p) j -> p k j", p=P))
            wv_s = wpool.tile([P, NKC, HD], BF16)
            nc.sync.dma_start(out=wv_s, in_=wvT[:, :].rearrange(
                "(k p) j -> p k j", p=P))

            cos_s = constp.tile([P, T], F32)
            sin_s = constp.tile([P, T], F32)
            nc.sync.dma_start(out=cos_s, in_=cost[:, :])
            nc.sync.dma_start(out=sin_s, in_=sint[:, :])

            rT_s = constp.tile([P, P], BF16)
            nc.sync.dma_start(out=rT_s, in_=rT[:, :])
            qb_s = constp.tile([P, HPC], F32)
            kb_s = constp.tile([P, 1], F32)
            vb_s = constp.tile([P, 1], F32)
            nc.sync.dma_start(out=qb_s, in_=qb[:, :])
            nc.sync.dma_start(out=kb_s, in_=kb[:, :])
            nc.sync.dma_start(out=vb_s, in_=vb[:, :])

            ones_s = constp.tile([P, P], BF16)
            nc.vector.memset(ones_s, 1.0)
            ident_s = constp.tile([P, P], BF16)
            make_identity(nc, ident_s)

            # persistent Q/K/V in rope-d transposed layout
            qT_s = qkvp.tile([P, HPC, T], BF16)   # [hd, head, t]
            kT_s = qkvp.tile([P, T], BF16)        # [hd, t]
            vN_s = qkvp.tile([P, TKC, HD], BF16)  # [tk%128, tk//128, hd]

            # ---- phases 1+2 interleaved per t-chunk ----------------------
            # attention for chunk c only needs projections from chunks <= c,
            # so it is emitted right after chunk n=c's projections; the
            # scheduler fills its exp-latency bubbles with the next chunk's
            # projection matmuls, and the chunk's all-gather fires early.
            for n in range(NT):
                ts = slice(n * 512, (n + 1) * 512)
                if 1 in phases:
                    # x for this t-chunk: four 8-k-chunk quarter tiles (the
                    # very first quarter arrives in two halves)
                    xtq = []
                    for q in range(4):
                        ks = slice(q * 8 * P, (q + 1) * 8 * P)
                        xq = xsp.tile([P, 8, 512], BF16, name=f"xt{n}_{q}",
                                      tag="xs")
                        eng = nc.gpsimd
                        if n == 0 and q == 0:
                            for ha, hb in ((0, 2), (2, 8)):
                                eng.dma_start(
                                    out=xq[:, ha:hb, :],
                                    in_=xTb[n, ha * P:hb * P, :].rearrange(
                                        "(k p) t -> p k t", p=P))
                        else:
                            eng.dma_start(
                                out=xq,
                                in_=xTb[n, ks, :].rearrange("(k p) t -> p k t",
                                                            p=P))
                        xtq.append(xq)

                    def xt_sl(k):
                        return xtq[k // 8][:, k % 8, :]

                    acc_tiles = []
                    for m in range(HPC + 2):  # 4 q-head tiles, k, v
                        pacc = accp.tile([P, 512], F32, name=f"pacc{n}_{m}",
                                         tag="acc")
                        acc_tiles.append(pacc)
                    # q-pass
                    for k in range(NKC):
                        for m in range(HPC):
                            nc.tensor.matmul(
                                acc_tiles[m],
                                lhsT=wo_s[:, k, m * P:(m + 1) * P],
                                rhs=xt_sl(k),
                                start=(k == 0),
                                stop=(k == NKC - 1),
                            )
                    # rope for the 4 q tiles
                    for m in range(HPC):
                        braw = stagep.tile([P, 512], BF16, name=f"braw{n}_{m}",
                                           tag="braw")
                        nc.scalar.add(braw, acc_tiles[m], qb_s[:, m:m + 1])
                        rot_ps = accp.tile([P, 512], F32, name=f"rot{n}_{m}",
                                           tag="acc")
                        nc.tensor.matmul(rot_ps, lhsT=rT_s, rhs=braw,
                                         start=True, stop=True)
                        dst = qT_s[:, m, ts]
                        tmp = stagep.tile([P, 512], F32, name=f"tmp{n}_{m}",
                                          tag="stage")
                        nc.vector.tensor_mul(tmp, rot_ps, sin_s[:, ts])
                        nc.vector.tensor_mul(dst, braw, cos_s[:, ts])
                        nc.vector.tensor_add(dst, dst, tmp)
                    # kv-pass
                    for k in range(NKC):
                        nc.tensor.matmul(
                            acc_tiles[HPC], lhsT=wk_s[:, k, :], rhs=xt_sl(k),
                            start=(k == 0), stop=(k == NKC - 1),
                        )
                        nc.tensor.matmul(
                            acc_tiles[HPC + 1], lhsT=wv_s[:, k, :], rhs=xt_sl(k),
                            start=(k == 0), stop=(k == NKC - 1),
                        )
                    # rope for the k tile
                    braw = stagep.tile([P, 512], BF16, name=f"brawk{n}",
                                       tag="braw")
                    nc.scalar.add(braw, acc_tiles[HPC], kb_s[:, 0:1])
                    rot_ps = accp.tile([P, 512], F32, name=f"rotk{n}", tag="acc")
                    nc.tensor.matmul(rot_ps, lhsT=rT_s, rhs=braw,
                                     start=True, stop=True)
                    tmp = stagep.tile([P, 512], F32, name=f"tmpk{n}", tag="stage")
                    nc.vector.tensor_mul(tmp, rot_ps, sin_s[:, ts])
                    nc.vector.tensor_mul(kT_s[:, ts], braw, cos_s[:, ts])
                    nc.vector.tensor_add(kT_s[:, ts], kT_s[:, ts], tmp)

                    # v: bias then transpose into natural layout
                    v_st = stagep.tile([P, 512], BF16, name=f"vst{n}", tag="braw")
                    nc.scalar.add(v_st, acc_tiles[HPC + 1], vb_s[:, 0:1])
                    for j in range(4):
                        vt_ps = accp.tile([P, P], BF16, name=f"vt{n}_{j}",
                                          tag="acc")
                        nc.tensor.transpose(vt_ps, v_st[:, j * P:(j + 1) * P],
                                            ident_s)
                        nc.scalar.copy(vN_s[:, n * 4 + j, :], vt_ps)

                if 2 not in phases:
                    continue
                # ---- attention for chunk c = n + its all-gather ----------
                c = n
                cs = slice(c * 512, (c + 1) * 512)
                act_ks = [k for k in range(TKC) if klass[k, c] != MSK_SKIP]
                add_ks = [k for k in act_ks if klass[k, c] == MSK_ADD]
                mtiles = {}
                for k in add_ks:
                    mt = attp.tile([P, 512], BF16, name=f"mt{c}_{k}", tag="msk",
                                   bufs=max(2, len(add_ks) + 1))
                    nc.sync.dma_start(out=mt, in_=maskTb[c, k * P:(k + 1) * P, :])
                    mtiles[k] = mt
                for h in range(HPC):
                    attn_ps = adp.tile([P, 512], F32, name=f"apv{c}_{h}",
                                       tag="attden")
                    den_ps = adp.tile([P, 512], F32, name=f"den{c}_{h}",
                                      tag="attden")
                    nact = len(act_ks)
                    for i, k in enumerate(act_ks):
                        # leading fully-masked tq columns contribute exactly 0
                        # after exp, so shrink the tile.  The first matmul of
                        # each accumulation group stays full width so
                        # start=True clears the whole psum bank.
                        off = 0 if i == 0 else int(col0[k, c])
                        qs = slice(c * 512 + off, (c + 1) * 512)
                        s_ps = sattp.tile([P, 512], F32, name=f"sps{c}_{h}_{k}",
                                          tag="satt")
                        nc.tensor.matmul(
                            s_ps[:, off:],
                            lhsT=kT_s[:, k * P:(k + 1) * P],
                            rhs=qT_s[:, h, qs],
                            start=True, stop=True,
                        )
                        if k in mtiles:
                            nc.vector.tensor_add(s_ps[:, off:], s_ps[:, off:],
                                                 mtiles[k][:, off:])
                        e_sb = attp.tile([P, 512], BF16, name=f"e{c}_{h}_{k}",
                                         tag="exp", bufs=6)
                        # exp(SCALE * s + mask): mask was pre-divided by
                        # SCALE on the host, so the add can happen upstream.
                        nc.scalar.activation(
                            e_sb[:, off:], s_ps[:, off:],
                            mybir.ActivationFunctionType.Exp, scale=SCALE)
                        nc.tensor.matmul(
                            attn_ps[:, off:], lhsT=vN_s[:, k, :],
                            rhs=e_sb[:, off:],
                            start=(i == 0), stop=(i == nact - 1),
                        )
                        nc.tensor.matmul(
                            den_ps[:, off:], lhsT=ones_s, rhs=e_sb[:, off:],
                            start=(i == 0), stop=(i == nact - 1),
                        )
                    rcp = attp.tile([P, 512], F32, name=f"rcp{c}_{h}", tag="rcp",
                                    bufs=2)
                    nc.vector.reciprocal(rcp, den_ps)
                    attn_sb = stagep.tile([P, 512], BF16, name=f"ao{c}_{h}",
                                          tag="braw")
                    nc.vector.tensor_mul(attn_sb, attn_ps, rcp)
                    nc.sync.dma_start(out=ag_in[c, h * P:(h + 1) * P, :],
                                      in_=attn_sb)
                if 25 in phases:
                    nc.gpsimd.collective_compute(
                        "AllGather",
                        mybir.AluOpType.bypass,
                        replica_groups=[list(range(NCORE))],
                        ins=[ag_in[c]],
                        outs=[ag_out[c]],
                    )

            # ---- phase 3: output projection ------------------------------
            for n in range(NT) if 3 in phases else []:
                rq_tiles = []
                for q in range(4):
                    ks = slice(q * 8 * P, (q + 1) * 8 * P)
                    rq = xsp.tile([P, 8, 512], BF16, name=f"r{n}_{q}", tag="xs")
                    nc.gpsimd.dma_start(
                        out=rq,
                        in_=ag_out[n, ks, :].rearrange("(k p) t -> p k t", p=P))
                    rq_tiles.append(rq)

                def r_sl(k):
                    return rq_tiles[k // 8][:, k % 8, :]

                o_acc = []
                for m in range(HPC):
                    po = accp.tile([P, 512], F32, name=f"oacc{n}_{m}", tag="acc")
                    o_acc.append(po)
                for k in range(NKC):
                    for m in range(HPC):
                        nc.tensor.matmul(
                            o_acc[m],
                            lhsT=wo_s[:, k, m * P:(m + 1) * P],
                            rhs=r_sl(k),
                            start=(k == 0),
                            stop=(k == NKC - 1),
                        )
                for m in range(HPC):
                    o_sb = stagep.tile([P, 512], F32, name=f"o{n}_{m}",
                                       tag="stage")
                    nc.scalar.add(o_sb, o_acc[m], qb_s[:, m:m + 1])
                    nc.sync.dma_start(out=outTb[n, m * P:(m + 1) * P, :],
                                      in_=o_sb)

    legalize_waits(nc)
    return nc


def _marshal_inputs(x, freqs_cos, freqs_sin, mask, wk_w, wk_b, wv_w, wv_b,
                    wo_w, wo_b):
    bf = ml_dtypes.bfloat16
    x = np.asarray(x, np.float32)
    mask = np.asarray(mask, np.float32)
    cos = np.asarray(freqs_cos, np.float32)
    sin = np.asarray(freqs_sin, np.float32)
    wk_w = np.asarray(wk_w, np.float32)
    wv_w = np.asarray(wv_w, np.float32)
    wo_w = np.asarray(wo_w, np.float32)
    wk_b = np.asarray(wk_b, np.float32)
    wv_b = np.asarray(wv_b, np.float32)
    wo_b = np.asarray(wo_b, np.float32)

    xT = x.reshape(T, DIM).T                       # (DIM, T)
    xTb = np.ascontiguousarray(
        xT.reshape(DIM, NT, 512).transpose(1, 0, 2).astype(bf))
    # mask applied on-device as exp(SCALE*s + SCALE*maskT): pre-divide, and
    # reblock (tq-chunk, tk, tq') so every mask tile DMA is contiguous
    maskT = mask.T / np.float32(SCALE)             # (tk, tq)
    maskTb = np.ascontiguousarray(
        maskT.reshape(T, NT, 512).transpose(1, 0, 2).astype(bf))

    cos2 = np.repeat(cos.T, 2, axis=0)  # (128, T): rows 2i,2i+1 = cos[:, i]
    sin2 = np.repeat(sin.T, 2, axis=0)

    # rotation matmul constant: out = R @ q with rot[2i] = -q[2i+1],
    # rot[2i+1] = q[2i]; lhsT layout (R transposed).
    RT = np.zeros((P, P), np.float32)
    idx = np.arange(0, P, 2)
    RT[idx + 1, idx] = -1.0
    RT[idx, idx + 1] = 1.0

    common = dict(
        xTb=xTb, maskTb=maskTb,
        cost=np.ascontiguousarray(cos2),
        sint=np.ascontiguousarray(sin2),
        rT=RT.astype(bf),
    )

    in_maps = []
    for cix in range(NCORE):
        jlo = cix * JPC
        klo = cix * HD
        m = dict(common)
        m["woT"] = np.ascontiguousarray(wo_w[jlo:jlo + JPC, :].T.astype(bf))
        m["wkT"] = np.ascontiguousarray(wk_w[klo:klo + HD, :].T.astype(bf))
        m["wvT"] = np.ascontiguousarray(wv_w[klo:klo + HD, :].T.astype(bf))
        m["qb"] = np.ascontiguousarray(wo_b[jlo:jlo + JPC].reshape(HPC, P).T)
        m["kb"] = np.ascontiguousarray(wk_b[klo:klo + HD].reshape(1, P).T)
        m["vb"] = np.ascontiguousarray(wv_b[klo:klo + HD].reshape(1, P).T)
        in_maps.append(m)
    return in_maps, mask


def run(inputs, trace=False):
    """Build, run on 8 cores, return (full_output, BassKernelResults)."""
    in_maps, mask = _marshal_inputs(
        inputs["x"], inputs["freqs_cos"], inputs["freqs_sin"], inputs["mask"],
        inputs["wk_w"], inputs["wk_b"], inputs["wv_w"], inputs["wv_b"],
        inputs["wo_w"], inputs["wo_b"])
    klass, col0 = _classify_mask(mask)
    nc = _build_module(klass, col0)
    res = run_bass_kernel_spmd(nc, in_maps, core_ids=list(range(NCORE)),
                               trace=trace)
    out = np.empty((DIM, T), np.float32)
    for cix in range(NCORE):
        ob = res.results[cix]["outTb"]          # (NT, JPC, 512)
        for n in range(NT):
            out[cix * JPC:(cix + 1) * JPC, n * 512:(n + 1) * 512] = ob[n]
    out = out.T  # (T, DIM)
    return np.ascontiguousarray(out[None, :, :]).astype(np.float32), res


def kernel(**inputs):
    out, _ = run(inputs, trace=False)
    return out



# revision 17
# speedup vs baseline: 1.3251x; 1.3251x over previous
"""Tensor-parallel GQA attention prefill (B=1, T=2048, D=4096, 32 q-heads /
8 kv-heads) for 8 Trainium2 NeuronCores.

Sharding: head-parallel.  Core c owns q-heads [4c, 4c+4) and kv-head c.
  phase 1: Q/K/V projections in transposed layout (head-dim on partitions),
           RoPE applied via a rotation-matmul + two table multiplies.
  phase 2: per-head attention with scores held transposed (tk on
           partitions); softmax denominators come from a ones-matmul;
           fully-masked tiles are skipped (host inspects the mask tensor);
           score matmuls are emitted two tiles ahead of their exp/AV
           consumers so the PE never stalls on activation latency.
  phase 3: contraction-sharded output projection: each core contracts its
           own 512 attention dims (4 heads x 128, straight out of SBUF)
           against wo columns, producing a full-height [4096, 512] partial
           per t-chunk; bias is folded into core 0's partial.
  phase 3.5: per-t-chunk ReduceScatter (add) sums the partials and hands
           core r the output-dim slice [512r, 512r+512), written directly
           to the kernel output; pipelined against later chunks' compute.

Matmul operands are bf16 (fp32 accumulation in PSUM); measured end-to-end
error vs the fp32 reference is ~5e-3 relative.

NOTE: faithful to the reference "bug" -- the q projection uses wo_w/wo_b.
"""

import numpy as np
import ml_dtypes

import bass_rust
import concourse.bass as bass
import concourse.mybir as mybir
import concourse.tile as tile
from concourse.bass_utils import run_bass_kernel_spmd
from concourse.masks import make_identity

# problem constants (self-contained; do not read spec.json)
DIM = 4096
NH = 32
NKV = 8
HD = 128
T = 2048
NCORE = 8
HPC = NH // NCORE      # 4 q heads per core
JPC = HPC * HD         # 512 contraction dims owned per core
P = 128
NT = T // 512          # 4 free-dim chunks of 512
NKC = DIM // P         # 32 contraction chunks in the projections
NOG = DIM // P         # 32 output-row groups in the out projection
TKC = T // P           # 16 tk chunks in attention
SCALE = 1.0 / float(np.sqrt(HD))

F32 = mybir.dt.float32
BF16 = mybir.dt.bfloat16

# mask tile classification
MSK_SKIP, MSK_ZERO, MSK_ADD = 0, 1, 2


def legalize_waits(nc, max_waits=1):
    """Hoist excess on_wait conditions onto preceding nop instructions.

    This walrus build rejects instructions carrying more than a couple of
    sync-wait commands; engines execute their queue in order, so a nop that
    waits immediately before the real instruction is equivalent.
    """
    n_new = 0
    for f in nc.m.functions:
        for bb in f.blocks:
            insts = bb.instructions
            new = []
            for ins in list(insts):
                si = ins.sync_info
                waits = list(si.on_wait) if si is not None and si.on_wait else []
                if len(waits) > max_waits:
                    hoist = waits[:-max_waits]
                    keep = waits[-max_waits:]
                    for j in range(0, len(hoist), max_waits):
                        chunk = hoist[j:j + max_waits]
                        nop = mybir.InstNoOp(
                            name=f"{ins.name}_hw{j}",
                            engine=ins.engine,
                            sync_info=bass_rust.SyncInfo(
                                on_wait=chunk, on_update=[]),
                        )
                        new.append(nop)
                        n_new += 1
                    ins.sync_info = bass_rust.SyncInfo(
                        on_wait=keep,
                        on_update=list(si.on_update) if si.on_update else [])
                new.append(ins)
            insts.clear()
            insts.extend(new)
    return n_new


def _classify_mask(mask):
    """Per (tk-chunk, tq-chunk-of-512) classification of the additive mask.

    Returns (klass, col0) where col0[k, c] is the first tq column (multiple
    of 128) of the chunk that is not fully masked -- matmuls/exp for the
    columns before it are skipped (their softmax weights are exactly 0).
    """
    klass = np.empty((TKC, NT), dtype=np.int32)
    col0 = np.zeros((TKC, NT), dtype=np.int32)
    for k in range(TKC):
        for c in range(NT):
            blk = mask[c * 512:(c + 1) * 512, k * P:(k + 1) * P]
            mx = float(blk.max())
            mn = float(blk.min())
            if mx < -80.0:
                klass[k, c] = MSK_SKIP
                continue
            if mx == 0.0 and mn == 0.0:
                klass[k, c] = MSK_ZERO
            else:
                klass[k, c] = MSK_ADD
            # leading fully-masked tq columns, rounded down to 128
            colmax = blk.max(axis=1)          # per-tq-row max over this tile
            nz = np.nonzero(colmax >= -80.0)[0]
            first = int(nz[0]) if len(nz) else 0
            first = (first // P) * P
            # only safe to skip if every column before `first` is fully masked
            if first > 0 and float(blk[:first].max()) < -80.0:
                col0[k, c] = first
    # never allow a fully-empty (all-skip) tq chunk; keep one tile live
    for c in range(NT):
        if all(klass[k, c] == MSK_SKIP for k in range(TKC)):
            klass[min(c * 4, TKC - 1), c] = MSK_ADD
    return klass, col0


def _build_module(klass, col0, phases=(1, 2, 3)):
    nc = bass.Bass()

    # inputs are pre-reblocked on the host so every DMA is contiguous
    xTb = nc.declare_dram_parameter("xTb", [NT, DIM, 512], BF16, isOutput=False)
    woT = nc.declare_dram_parameter("woT", [DIM, JPC], BF16, isOutput=False)
    woJ = nc.declare_dram_parameter("woJ", [P, HPC, DIM], BF16, isOutput=False)
    wkT = nc.declare_dram_parameter("wkT", [DIM, HD], BF16, isOutput=False)
    wvT = nc.declare_dram_parameter("wvT", [DIM, HD], BF16, isOutput=False)
    # packed biases: [qb(HPC) | kb(1) | vb(1) | ob(NOG)] -- one DMA
    cb = nc.declare_dram_parameter("cb", [P, HPC + 2 + NOG], F32,
                                   isOutput=False)
    maskTb = nc.declare_dram_parameter("maskTb", [NT, T, 512], BF16,
                                       isOutput=False)
    cost = nc.declare_dram_parameter("cost", [P, T], F32, isOutput=False)
    sint = nc.declare_dram_parameter("sint", [P, T], F32, isOutput=False)
    rT = nc.declare_dram_parameter("rT", [P, P], BF16, isOutput=False)
    outTb = nc.declare_dram_parameter("outTb", [NT, JPC, 512], BF16,
                                      isOutput=True)

    rs_in = nc.dram_tensor("rs_in", [NT, DIM, 512], BF16)
    rs_out = nc.dram_tensor("rs_out", [NT, JPC, 512], BF16)

    with tile.TileContext(nc) as tc:
        with (
            tc.tile_pool(name="wpool", bufs=1) as wpool,
            tc.tile_pool(name="const", bufs=1) as constp,
            tc.tile_pool(name="qkv", bufs=1) as qkvp,
            tc.tile_pool(name="xs", bufs=6) as xsp,
            tc.tile_pool(name="stage", bufs=4) as stagep,
            tc.tile_pool(name="att", bufs=4) as attp,
            tc.tile_pool(name="oout", bufs=6) as ooutp,
            tc.tile_pool(name="acc", bufs=3, space="PSUM") as accp,
            tc.tile_pool(name="satt", bufs=3, space="PSUM") as sattp,
            tc.tile_pool(name="attden", bufs=2, space="PSUM") as adp,
        ):
            # ---- chunk-0 x tiles + wo pieces, interleaved: the first
            # k-loop consumes both streams in ktile order at ~300 GB/s, so
            # neither may monopolize the DMA device -----------------------
            xtq0 = [xsp.tile([P, 8, 512], BF16, name=f"xt0_{q}", tag="xs")
                    for q in range(4)]
            wo_s = wpool.tile([P, NKC, JPC], BF16)
            x_parts = []
            for q in range(4):
                lim = (0, 2, 8) if q == 0 else (0, 4, 8)
                for ha, hb in zip(lim[:-1], lim[1:]):
                    x_parts.append((q, ha, hb))
            wo_parts = [(0, 1), (1, 2), (2, 4), (4, 8), (8, 12), (12, 16),
                        (16, 20), (20, 24), (24, 28), (28, 32)]
            for i in range(max(len(x_parts), len(wo_parts))):
                if i < len(x_parts):
                    q, ha, hb = x_parts[i]
                    ks = slice(q * 8 * P, (q + 1) * 8 * P)
                    nc.gpsimd.dma_start(
                        out=xtq0[q][:, ha:hb, :],
                        in_=xTb[0, ks][ha * P:hb * P, :].rearrange(
                            "(k p) t -> p k t", p=P))
                if i < len(wo_parts):
                    klo, khi = wo_parts[i]
                    nc.sync.dma_start(
                        out=wo_s[:, klo:khi, :],
                        in_=woT[klo * P:khi * P, :].rearrange(
                            "(k p) j -> p k j", p=P))

            wk_s = wpool.tile([P, NKC, HD], BF16)
            nc.sync.dma_start(out=wk_s, in_=wkT[:, :].rearrange(
                "(k p) j -> p k j", p=P))
            wv_s = wpool.tile([P, NKC, HD], BF16)
            nc.sync.dma_start(out=wv_s, in_=wvT[:, :].rearrange(
                "(k p) j -> p k j", p=P))
            # wo in contraction-on-own-dims layout for the output projection;
            # loaded piecewise after chunk 0's x tiles (first needed by the
            # chunk-0 output projection, much later than x/wo/cos/sin).
            woJ_s = wpool.tile([P, HPC, DIM], BF16)

            cos_s = constp.tile([P, T], F32)
            sin_s = constp.tile([P, T], F32)
            nc.sync.dma_start(out=cos_s, in_=cost[:, :])
            nc.sync.dma_start(out=sin_s, in_=sint[:, :])

            rT_s = constp.tile([P, P], BF16)
            nc.sync.dma_start(out=rT_s, in_=rT[:, :])
            cb_s = constp.tile([P, HPC + 2 + NOG], F32)
            nc.sync.dma_start(out=cb_s, in_=cb[:, :])
            qb_s = cb_s[:, 0:HPC]
            kb_s = cb_s[:, HPC:HPC + 1]
            vb_s = cb_s[:, HPC + 1:HPC + 2]
            ob_s = cb_s[:, HPC + 2:]

            ones_s = constp.tile([P, P], BF16)
            nc.vector.memset(ones_s, 1.0)
            ident_s = constp.tile([P, P], BF16)
            make_identity(nc, ident_s)

            # persistent Q/K/V in rope-d transposed layout
            qT_s = qkvp.tile([P, HPC, T], BF16)   # [hd, head, t]
            kT_s = qkvp.tile([P, T], BF16)        # [hd, t]
            vN_s = qkvp.tile([P, TKC, HD], BF16)  # [tk%128, tk//128, hd]

            # ---- phases 1+2+3 interleaved per t-chunk --------------------
            # attention for chunk c only needs projections from chunks <= c,
            # so it is emitted right after chunk n=c's projections; the
            # output-projection partial for chunk c follows immediately
            # (it only needs this core's own attention output), and its
            # ReduceScatter fires while chunk c+1 computes.
            for n in range(NT):
                ts = slice(n * 512, (n + 1) * 512)
                if 1 in phases:
                    # x for this t-chunk (chunk 0's tiles were loaded up top)
                    if n == 0:
                        xtq = xtq0
                        for jt in range(HPC):
                            nc.sync.dma_start(out=woJ_s[:, jt, :],
                                              in_=woJ[:, jt, :])
                    else:
                        xtq = []
                        for q in range(4):
                            ks = slice(q * 8 * P, (q + 1) * 8 * P)
                            xq = xsp.tile([P, 8, 512], BF16, name=f"xt{n}_{q}",
                                          tag="xs")
                            nc.gpsimd.dma_start(
                                out=xq,
                                in_=xTb[n, ks, :].rearrange("(k p) t -> p k t",
                                                            p=P))
                            xtq.append(xq)

                    def xt_sl(k):
                        return xtq[k // 8][:, k % 8, :]

                    # q-pass: one head at a time to keep PSUM pressure low;
                    # each head's rope rot-matmul is deferred into the next
                    # head's k-loop so the PE never waits on the bias add.
                    braws = {}

                    def emit_rope_q(m):
                        braw = braws.pop(m)
                        rot_ps = accp.tile([P, 512], F32, name=f"rot{n}_{m}",
                                           tag="acc")
                        nc.tensor.matmul(rot_ps, lhsT=rT_s, rhs=braw,
                                         start=True, stop=True)
                        dst = qT_s[:, m, ts]
                        tmp = stagep.tile([P, 512], F32, name=f"tmp{n}_{m}",
                                          tag="stage")
                        nc.vector.tensor_mul(tmp, rot_ps, sin_s[:, ts])
                        nc.vector.tensor_mul(dst, braw, cos_s[:, ts])
                        nc.vector.tensor_add(dst, dst, tmp)

                    for m in range(HPC):
                        pacc = accp.tile([P, 512], F32, name=f"pacc{n}_{m}",
                                         tag="acc")
                        for k in range(NKC):
                            nc.tensor.matmul(
                                pacc,
                                lhsT=wo_s[:, k, m * P:(m + 1) * P],
                                rhs=xt_sl(k),
                                start=(k == 0),
                                stop=(k == NKC - 1),
                            )
                        braw = stagep.tile([P, 512], BF16, name=f"braw{n}_{m}",
                                           tag="braw")
                        nc.scalar.add(braw, pacc, qb_s[:, m:m + 1])
                        braws[m] = braw
                        if m > 0:
                            emit_rope_q(m - 1)
                    # kv-pass (covers the last q head's bias-add latency)
                    kacc = accp.tile([P, 512], F32, name=f"kacc{n}", tag="acc")
                    vacc = accp.tile([P, 512], F32, name=f"vacc{n}", tag="acc")
                    for k in range(NKC):
                        nc.tensor.matmul(
                            kacc, lhsT=wk_s[:, k, :], rhs=xt_sl(k),
                            start=(k == 0), stop=(k == NKC - 1),
                        )
                        nc.tensor.matmul(
                            vacc, lhsT=wv_s[:, k, :], rhs=xt_sl(k),
                            start=(k == 0), stop=(k == NKC - 1),
                        )
                    emit_rope_q(HPC - 1)
                    # rope for the k tile (bias add queued behind the q ones)
                    braw = stagep.tile([P, 512], BF16, name=f"brawk{n}",
                                       tag="braw")
                    nc.scalar.add(braw, kacc, kb_s[:, 0:1])
                    rot_ps = accp.tile([P, 512], F32, name=f"rotk{n}", tag="acc")
                    nc.tensor.matmul(rot_ps, lhsT=rT_s, rhs=braw,
                                     start=True, stop=True)
                    tmp = stagep.tile([P, 512], F32, name=f"tmpk{n}", tag="stage")
                    nc.vector.tensor_mul(tmp, rot_ps, sin_s[:, ts])
                    nc.vector.tensor_mul(kT_s[:, ts], braw, cos_s[:, ts])
                    nc.vector.tensor_add(kT_s[:, ts], kT_s[:, ts], tmp)

                    # v: bias then transpose into natural layout
                    v_st = stagep.tile([P, 512], BF16, name=f"vst{n}", tag="braw")
                    nc.scalar.add(v_st, vacc, vb_s[:, 0:1])
                    for j in range(4):
                        vt_ps = accp.tile([P, P], BF16, name=f"vt{n}_{j}",
                                          tag="acc")
                        nc.tensor.transpose(vt_ps, v_st[:, j * P:(j + 1) * P],
                                            ident_s)
                        nc.scalar.copy(vN_s[:, n * 4 + j, :], vt_ps)

                if 2 not in phases:
                    continue
                # ---- attention for chunk c = n ---------------------------
                c = n
                act_ks = [k for k in range(TKC) if klass[k, c] != MSK_SKIP]
                add_ks = [k for k in act_ks if klass[k, c] == MSK_ADD]
                mtiles = {}
                for k in add_ks:
                    mt = attp.tile([P, 512], BF16, name=f"mt{c}_{k}", tag="msk",
                                   bufs=max(2, len(add_ks) + 1))
                    nc.sync.dma_start(out=mt, in_=maskTb[c, k * P:(k + 1) * P, :])
                    mtiles[k] = mt
                attn_out = []
                for h in range(HPC):
                    attn_ps = adp.tile([P, 512], F32, name=f"apv{c}_{h}",
                                       tag="attden")
                    den_ps = adp.tile([P, 512], F32, name=f"den{c}_{h}",
                                      tag="attden")
                    nact = len(act_ks)

                    # score matmuls run LOOKAHEAD tiles ahead of their
                    # exp/AV consumers so the PE stays busy through the
                    # activation-engine latency of each tile.
                    LOOKAHEAD = 2
                    s_tiles = {}

                    def emit_score(i):
                        k = act_ks[i]
                        off = 0 if i == 0 else int(col0[k, c])
                        qs = slice(c * 512 + off, (c + 1) * 512)
                        s_ps = sattp.tile([P, 512], F32, name=f"sps{c}_{h}_{k}",
                                          tag="satt")
                        nc.tensor.matmul(
                            s_ps[:, off:],
                            lhsT=kT_s[:, k * P:(k + 1) * P],
                            rhs=qT_s[:, h, qs],
                            start=True, stop=True,
                        )
                        if k in mtiles:
                            nc.vector.tensor_add(s_ps[:, off:], s_ps[:, off:],
                                                 mtiles[k][:, off:])
                        s_tiles[i] = (s_ps, off)

                    def emit_consume(i):
                        k = act_ks[i]
                        s_ps, off = s_tiles.pop(i)
                        e_sb = attp.tile([P, 512], BF16, name=f"e{c}_{h}_{k}",
                                         tag="exp", bufs=5)
                        # exp(SCALE * s + mask): mask was pre-divided by
                        # SCALE on the host, so the add can happen upstream.
                        nc.scalar.activation(
                            e_sb[:, off:], s_ps[:, off:],
                            mybir.ActivationFunctionType.Exp, scale=SCALE)
                        nc.tensor.matmul(
                            attn_ps[:, off:], lhsT=vN_s[:, k, :],
                            rhs=e_sb[:, off:],
                            start=(i == 0), stop=(i == nact - 1),
                        )
                        nc.tensor.matmul(
                            den_ps[:, off:], lhsT=ones_s, rhs=e_sb[:, off:],
                            start=(i == 0), stop=(i == nact - 1),
                        )

                    for i in range(nact):
                        emit_score(i)
                        if i >= LOOKAHEAD:
                            emit_consume(i - LOOKAHEAD)
                    for i in range(max(0, nact - LOOKAHEAD), nact):
                        emit_consume(i)

                    rcp = attp.tile([P, 512], F32, name=f"rcp{c}_{h}", tag="rcp",
                                    bufs=2)
                    nc.vector.reciprocal(rcp, den_ps)
                    attn_sb = ooutp.tile([P, 512], BF16, name=f"ao{c}_{h}",
                                         tag="aout", bufs=5)
                    nc.vector.tensor_mul(attn_sb, attn_ps, rcp)
                    attn_out.append(attn_sb)

                if 3 not in phases:
                    continue
                # ---- phase 3: output-projection partial + ReduceScatter --
                # partial[out, tq] = sum_h woJ[:, h, out].T @ attn_out[h];
                # full 4096-row partial, summed across cores by the RS.
                # Groups run in pairs (kt interleaved) so the PE has work
                # while head 3's attention output finishes on the DVE, with
                # a 5-deep PSUM rotation across the acc + attden pools.
                def o_alloc(m, pool):
                    tg = "acc" if pool is accp else "attden"
                    return pool.tile([P, 512], F32, name=f"oacc{n}_{m}",
                                     tag=tg)

                def o_mm(m, po, kt):
                    nc.tensor.matmul(
                        po,
                        lhsT=woJ_s[:, kt, m * P:(m + 1) * P],
                        rhs=attn_out[kt],
                        start=(kt == 0),
                        stop=(kt == HPC - 1),
                    )

                def o_finish(m, po):
                    o_sb = ooutp.tile([P, 512], BF16, name=f"o{n}_{m}",
                                      tag="osb", bufs=4)
                    # bias is nonzero only on core 0 so the RS adds it once
                    # (gpsimd cannot read PSUM, so alternate Act/DVE)
                    if m % 2 == 0:
                        nc.scalar.add(o_sb, po, ob_s[:, m:m + 1])
                    else:
                        nc.vector.tensor_scalar_add(o_sb, po, ob_s[:, m:m + 1])
                    nc.sync.dma_start(out=rs_in[n, m * P:(m + 1) * P, :],
                                      in_=o_sb)

                # groups 0-2 (accp) prefill heads 0-2 while head 3's output
                # finishes on the DVE; the adp-pool groups follow.
                pos3 = [o_alloc(m, accp) for m in range(3)]
                for kt in range(HPC - 1):
                    for m in range(3):
                        o_mm(m, pos3[m], kt)
                for m in range(3):
                    o_mm(m, pos3[m], HPC - 1)
                    o_finish(m, pos3[m])
                opools = (adp, adp, accp, accp, accp)
                for mp in range(3, NOG, 2):
                    ms = [m for m in (mp, mp + 1) if m < NOG]
                    pos = [o_alloc(m, opools[(m - 3) % 5]) for m in ms]
                    for kt in range(HPC):
                        for i, m in enumerate(ms):
                            o_mm(m, pos[i], kt)
                    for i, m in enumerate(ms):
                        o_finish(m, pos[i])
                nc.gpsimd.collective_compute(
                    "ReduceScatter",
                    mybir.AluOpType.add,
                    replica_groups=[list(range(NCORE))],
                    ins=[rs_in[n]],
                    outs=[rs_out[n]],
                )
                nc.gpsimd.dma_start(out=outTb[n], in_=rs_out[n])

    legalize_waits(nc)
    return nc


def _marshal_inputs(x, freqs_cos, freqs_sin, mask, wk_w, wk_b, wv_w, wv_b,
                    wo_w, wo_b):
    bf = ml_dtypes.bfloat16
    x = np.asarray(x, np.float32)
    mask = np.asarray(mask, np.float32)
    cos = np.asarray(freqs_cos, np.float32)
    sin = np.asarray(freqs_sin, np.float32)
    wk_w = np.asarray(wk_w, np.float32)
    wv_w = np.asarray(wv_w, np.float32)
    wo_w = np.asarray(wo_w, np.float32)
    wk_b = np.asarray(wk_b, np.float32)
    wv_b = np.asarray(wv_b, np.float32)
    wo_b = np.asarray(wo_b, np.float32)

    xT = x.reshape(T, DIM).T                       # (DIM, T)
    xTb = np.ascontiguousarray(
        xT.reshape(DIM, NT, 512).transpose(1, 0, 2).astype(bf))
    # mask applied on-device as exp(SCALE*s + SCALE*maskT): pre-divide, and
    # reblock (tq-chunk, tk, tq') so every mask tile DMA is contiguous
    maskT = mask.T / np.float32(SCALE)             # (tk, tq)
    maskTb = np.ascontiguousarray(
        maskT.reshape(T, NT, 512).transpose(1, 0, 2).astype(bf))

    cos2 = np.repeat(cos.T, 2, axis=0)  # (128, T): rows 2i,2i+1 = cos[:, i]
    sin2 = np.repeat(sin.T, 2, axis=0)

    # rotation matmul constant: out = R @ q with rot[2i] = -q[2i+1],
    # rot[2i+1] = q[2i]; lhsT layout (R transposed).
    RT = np.zeros((P, P), np.float32)
    idx = np.arange(0, P, 2)
    RT[idx + 1, idx] = -1.0
    RT[idx, idx + 1] = 1.0

    common = dict(
        xTb=xTb, maskTb=maskTb,
        cost=np.ascontiguousarray(cos2),
        sint=np.ascontiguousarray(sin2),
        rT=RT.astype(bf),
    )

    in_maps = []
    for cix in range(NCORE):
        jlo = cix * JPC
        klo = cix * HD
        m = dict(common)
        m["woT"] = np.ascontiguousarray(wo_w[jlo:jlo + JPC, :].T.astype(bf))
        # contraction-sharded wo for the output projection:
        # woJ[p, jt, out] = wo_w[out, jlo + jt*128 + p]
        m["woJ"] = np.ascontiguousarray(
            wo_w[:, jlo:jlo + JPC].T.reshape(HPC, P, DIM)
            .transpose(1, 0, 2).astype(bf))
        m["wkT"] = np.ascontiguousarray(wk_w[klo:klo + HD, :].T.astype(bf))
        m["wvT"] = np.ascontiguousarray(wv_w[klo:klo + HD, :].T.astype(bf))
        qb_ = wo_b[jlo:jlo + JPC].reshape(HPC, P).T
        kb_ = wk_b[klo:klo + HD].reshape(1, P).T
        vb_ = wv_b[klo:klo + HD].reshape(1, P).T
        # out-projection bias, added once via core 0's partial
        if cix == 0:
            ob_ = wo_b.reshape(NOG, P).T
        else:
            ob_ = np.zeros((P, NOG), np.float32)
        m["cb"] = np.ascontiguousarray(
            np.concatenate([qb_, kb_, vb_, ob_], axis=1).astype(np.float32))
        in_maps.append(m)
    return in_maps, mask


def run(inputs, trace=False):
    """Build, run on 8 cores, return (full_output, BassKernelResults)."""
    in_maps, mask = _marshal_inputs(
        inputs["x"], inputs["freqs_cos"], inputs["freqs_sin"], inputs["mask"],
        inputs["wk_w"], inputs["wk_b"], inputs["wv_w"], inputs["wv_b"],
        inputs["wo_w"], inputs["wo_b"])
    klass, col0 = _classify_mask(mask)
    nc = _build_module(klass, col0)
    res = run_bass_kernel_spmd(nc, in_maps, core_ids=list(range(NCORE)),
                               trace=trace)
    out = np.empty((DIM, T), np.float32)
    for cix in range(NCORE):
        ob = res.results[cix]["outTb"]          # (NT, JPC, 512) bf16 slices
        for n in range(NT):
            out[cix * JPC:(cix + 1) * JPC, n * 512:(n + 1) * 512] = \
                ob[n].astype(np.float32)
    out = out.T  # (T, DIM)
    return np.ascontiguousarray(out[None, :, :]).astype(np.float32), res


def kernel(**inputs):
    out, _ = run(inputs, trace=False)
    return out


# revision 32
# speedup vs baseline: 1.5330x; 1.1569x over previous
"""Tensor-parallel GQA attention prefill (B=1, T=2048, D=4096, 32 q-heads /
8 kv-heads) for 8 Trainium2 NeuronCores.

Sharding: head-parallel.  Core c owns q-heads [4c, 4c+4) and kv-head c.
  phase 1: Q/K/V projections in transposed layout (head-dim on partitions),
           RoPE applied via a rotation-matmul + two table multiplies.
  phase 2: per-head attention with scores held transposed (tk on
           partitions); softmax denominators come from a ones-matmul;
           fully-masked tiles are skipped (host inspects the mask tensor);
           score matmuls are emitted two tiles ahead of their exp/AV
           consumers so the PE never stalls on activation latency.
  phase 3: contraction-sharded output projection: each core contracts its
           own 512 attention dims (4 heads x 128, straight out of SBUF)
           against wo columns, producing a full-height [4096, 512] partial
           per t-chunk; bias is folded into core 0's partial.
  phase 3.5: per-t-chunk ReduceScatter (add) sums the partials and hands
           core r the output-dim slice [512r, 512r+512), written directly
           to the kernel output; pipelined against later chunks' compute.

Matmul operands are bf16 (fp32 accumulation in PSUM); measured end-to-end
error vs the fp32 reference is ~5e-3 relative.

NOTE: faithful to the reference "bug" -- the q projection uses wo_w/wo_b.
"""

import numpy as np
import ml_dtypes

import bass_rust
import concourse.bass as bass
import concourse.mybir as mybir
import concourse.tile as tile
from concourse.bass_utils import run_bass_kernel_spmd
from concourse.masks import make_identity

# problem constants (self-contained; do not read spec.json)
DIM = 4096
NH = 32
NKV = 8
HD = 128
T = 2048
NCORE = 8
HPC = NH // NCORE      # 4 q heads per core
JPC = HPC * HD         # 512 contraction dims owned per core
P = 128
NT = T // 512          # 4 free-dim chunks of 512
NKC = DIM // P         # 32 contraction chunks in the projections
NOG = DIM // P         # 32 output-row groups in the out projection
TKC = T // P           # 16 tk chunks in attention
SCALE = 1.0 / float(np.sqrt(HD))

F32 = mybir.dt.float32
BF16 = mybir.dt.bfloat16
FP8 = mybir.dt.float8e4
DRSW = mybir.MatmulPerfMode.DoubleRowSwInterleave

# fp8 error-compensated projections: w ~ S_W*w -> hi+lo fp8, x -> hi+lo fp8;
# x@w ~ (xh@wh + xl@wh + xh@wl)/S_W with fp32 PSUM accumulation.  DoubleRow
# processes two 128-ktile slots per instruction at 0.5 cycles/row, so the
# three terms cost 0.75x the bf16 schedule at slightly BETTER accuracy
# (the dropped xl@wl term is ~7e-4 relative).
S_W = 256.0            # weight scale (keeps fp8 out of the subnormal range)
S_A = 32.0             # attention-output scale for the out-projection rhs
NPR = NKC // 2         # 16 contraction ktile-pairs in the projections

# mask tile classification
MSK_SKIP, MSK_ZERO, MSK_ADD = 0, 1, 2


def legalize_waits(nc, max_waits=1):
    """Hoist excess on_wait conditions onto preceding nop instructions.

    This walrus build rejects instructions carrying more than a couple of
    sync-wait commands; engines execute their queue in order, so a nop that
    waits immediately before the real instruction is equivalent.
    """
    n_new = 0
    for f in nc.m.functions:
        for bb in f.blocks:
            insts = bb.instructions
            new = []
            for ins in list(insts):
                si = ins.sync_info
                waits = list(si.on_wait) if si is not None and si.on_wait else []
                if len(waits) > max_waits:
                    hoist = waits[:-max_waits]
                    keep = waits[-max_waits:]
                    for j in range(0, len(hoist), max_waits):
                        chunk = hoist[j:j + max_waits]
                        nop = mybir.InstNoOp(
                            name=f"{ins.name}_hw{j}",
                            engine=ins.engine,
                            sync_info=bass_rust.SyncInfo(
                                on_wait=chunk, on_update=[]),
                        )
                        new.append(nop)
                        n_new += 1
                    ins.sync_info = bass_rust.SyncInfo(
                        on_wait=keep,
                        on_update=list(si.on_update) if si.on_update else [])
                new.append(ins)
            insts.clear()
            insts.extend(new)
    return n_new


def _classify_mask(mask):
    """Per (tk-chunk, tq-chunk-of-512) classification of the additive mask.

    Returns (klass, col0) where col0[k, c] is the first tq column (multiple
    of 128) of the chunk that is not fully masked -- matmuls/exp for the
    columns before it are skipped (their softmax weights are exactly 0).
    """
    klass = np.empty((TKC, NT), dtype=np.int32)
    col0 = np.zeros((TKC, NT), dtype=np.int32)
    for k in range(TKC):
        for c in range(NT):
            blk = mask[c * 512:(c + 1) * 512, k * P:(k + 1) * P]
            mx = float(blk.max())
            mn = float(blk.min())
            if mx < -80.0:
                klass[k, c] = MSK_SKIP
                continue
            if mx == 0.0 and mn == 0.0:
                klass[k, c] = MSK_ZERO
            else:
                klass[k, c] = MSK_ADD
            # leading fully-masked tq columns, rounded down to 128
            colmax = blk.max(axis=1)          # per-tq-row max over this tile
            nz = np.nonzero(colmax >= -80.0)[0]
            first = int(nz[0]) if len(nz) else 0
            first = (first // P) * P
            # only safe to skip if every column before `first` is fully masked
            if first > 0 and float(blk[:first].max()) < -80.0:
                col0[k, c] = first
    # never allow a fully-empty (all-skip) tq chunk; keep one tile live
    for c in range(NT):
        if all(klass[k, c] == MSK_SKIP for k in range(TKC)):
            klass[min(c * 4, TKC - 1), c] = MSK_ADD
    return klass, col0


def _build_module(klass, col0, phases=(1, 2, 3)):
    nc = bass.Bass()

    # inputs are pre-reblocked on the host so every DMA is contiguous;
    # weights arrive as fp8 hi/lo pairs in DoubleRowSwInterleave layout
    # ([p, ktile-pair, m-block, 256] with A/B slots interleaved per column
    # and columns reversed within each 128-block).
    xh8 = nc.declare_dram_parameter("xh8", [NT, DIM, 512], FP8, isOutput=False)
    xl8 = nc.declare_dram_parameter("xl8", [NT, DIM, 512], FP8, isOutput=False)
    woh = nc.declare_dram_parameter("woh", [P, NPR, HPC, 2 * P], FP8,
                                    isOutput=False)
    wol = nc.declare_dram_parameter("wol", [P, NPR, HPC, 2 * P], FP8,
                                    isOutput=False)
    wkh = nc.declare_dram_parameter("wkh", [P, NPR, 1, 2 * P], FP8,
                                    isOutput=False)
    wkl = nc.declare_dram_parameter("wkl", [P, NPR, 1, 2 * P], FP8,
                                    isOutput=False)
    wvh = nc.declare_dram_parameter("wvh", [P, NPR, 1, 2 * P], FP8,
                                    isOutput=False)
    wvl = nc.declare_dram_parameter("wvl", [P, NPR, 1, 2 * P], FP8,
                                    isOutput=False)
    wjh = nc.declare_dram_parameter("wjh", [P, HPC // 2, NOG, 2 * P], FP8,
                                    isOutput=False)
    wjl = nc.declare_dram_parameter("wjl", [P, HPC // 2, NOG, 2 * P], FP8,
                                    isOutput=False)
    # packed biases: [qb(HPC) | kb(1) | vb(1) | ob(NOG)] -- one DMA
    cb = nc.declare_dram_parameter("cb", [P, HPC + 2 + NOG], F32,
                                   isOutput=False)
    maskTb = nc.declare_dram_parameter("maskTb", [NT, T, 512], BF16,
                                       isOutput=False)
    cost = nc.declare_dram_parameter("cost", [P, T], F32, isOutput=False)
    sint = nc.declare_dram_parameter("sint", [P, T], F32, isOutput=False)
    rT = nc.declare_dram_parameter("rT", [P, P], BF16, isOutput=False)
    outTb = nc.declare_dram_parameter("outTb", [NT, JPC, 512], BF16,
                                      isOutput=True)

    rs_in = nc.dram_tensor("rs_in", [NT, DIM, 512], BF16)
    rs_out = nc.dram_tensor("rs_out", [NT, JPC, 512], BF16)

    with tile.TileContext(nc) as tc:
        with (
            tc.tile_pool(name="wpool", bufs=1) as wpool,
            tc.tile_pool(name="const", bufs=1) as constp,
            tc.tile_pool(name="qkv", bufs=1) as qkvp,
            tc.tile_pool(name="xs", bufs=5) as xsp,
            tc.tile_pool(name="stage", bufs=3) as stagep,
            tc.tile_pool(name="att", bufs=4) as attp,
            tc.tile_pool(name="oout", bufs=6) as ooutp,
            tc.tile_pool(name="acc", bufs=3, space="PSUM") as accp,
            tc.tile_pool(name="satt", bufs=3, space="PSUM") as sattp,
            tc.tile_pool(name="attden", bufs=2, space="PSUM") as adp,
        ):
            # ---- chunk-0 x-hi tiles + wo-hi pieces, interleaved: the first
            # k-loop's A-terms consume both streams in ktile order, so
            # neither may monopolize the DMA device; the lo streams follow
            # (their B/C terms run after each head's A accumulation) -------
            xhq0 = [xsp.tile([P, 8, 512], FP8, name=f"xh0_{q}", tag="xh")
                    for q in range(4)]
            xlq0 = [xsp.tile([P, 8, 512], FP8, name=f"xl0_{q}", tag="xl")
                    for q in range(4)]
            woh_s = wpool.tile([P, NPR, HPC, 2 * P], FP8)
            wol_s = wpool.tile([P, NPR, HPC, 2 * P], FP8)
            x_parts = []
            for q in range(4):
                lim = (0, 2, 8) if q == 0 else (0, 4, 8)
                for ha, hb in zip(lim[:-1], lim[1:]):
                    x_parts.append((q, ha, hb))
            woh_parts = [(0, 1), (1, 2), (2, 3), (3, 4), (4, 6), (6, 8),
                         (8, 10), (10, 12), (12, 14), (14, 16)]
            for i in range(max(len(x_parts), len(woh_parts))):
                if i < len(x_parts):
                    q, ha, hb = x_parts[i]
                    ks = slice(q * 8 * P, (q + 1) * 8 * P)
                    nc.gpsimd.dma_start(
                        out=xhq0[q][:, ha:hb, :],
                        in_=xh8[0, ks][ha * P:hb * P, :].rearrange(
                            "(k p) t -> p k t", p=P))
                if i < len(woh_parts):
                    plo, phi = woh_parts[i]
                    nc.sync.dma_start(out=woh_s[:, plo:phi],
                                      in_=woh[:, plo:phi])
            # lo streams for chunk 0 (B-terms start ~2us into head 0)
            for q in range(4):
                ks = slice(q * 8 * P, (q + 1) * 8 * P)
                nc.gpsimd.dma_start(
                    out=xlq0[q],
                    in_=xl8[0, ks, :].rearrange("(k p) t -> p k t", p=P))
                nc.sync.dma_start(out=wol_s[:, 4 * q:4 * q + 4],
                                  in_=wol[:, 4 * q:4 * q + 4])

            wkh_s = wpool.tile([P, NPR, 1, 2 * P], FP8)
            wvh_s = wpool.tile([P, NPR, 1, 2 * P], FP8)
            wkl_s = wpool.tile([P, NPR, 1, 2 * P], FP8)
            wvl_s = wpool.tile([P, NPR, 1, 2 * P], FP8)
            nc.sync.dma_start(out=wkh_s, in_=wkh[:, :, :, :])
            nc.sync.dma_start(out=wvh_s, in_=wvh[:, :, :, :])
            nc.sync.dma_start(out=wkl_s, in_=wkl[:, :, :, :])
            nc.sync.dma_start(out=wvl_s, in_=wvl[:, :, :, :])
            # out-projection weights, loaded piecewise after chunk 0's x
            # (first needed by the chunk-0 output projection, much later)
            wjh_s = wpool.tile([P, HPC // 2, NOG, 2 * P], FP8)
            wjl_s = wpool.tile([P, HPC // 2, NOG, 2 * P], FP8)

            cos_s = constp.tile([P, T], F32)
            sin_s = constp.tile([P, T], F32)
            nc.sync.dma_start(out=cos_s, in_=cost[:, :])
            nc.sync.dma_start(out=sin_s, in_=sint[:, :])

            rT_s = constp.tile([P, P], BF16)
            nc.sync.dma_start(out=rT_s, in_=rT[:, :])
            cb_s = constp.tile([P, HPC + 2 + NOG], F32)
            nc.sync.dma_start(out=cb_s, in_=cb[:, :])
            qb_s = cb_s[:, 0:HPC]
            kb_s = cb_s[:, HPC:HPC + 1]
            vb_s = cb_s[:, HPC + 1:HPC + 2]
            ob_s = cb_s[:, HPC + 2:]

            # den accumulates sum(e)/S_A so attn_f = attn_ps * rcp comes out
            # pre-scaled by S_A for the fp8 split (1/32 is exact in bf16)
            ones_s = constp.tile([P, P], BF16)
            nc.vector.memset(ones_s, 1.0 / S_A)
            ident_s = constp.tile([P, P], BF16)
            make_identity(nc, ident_s)

            # persistent Q/K/V in rope-d transposed layout
            qT_s = qkvp.tile([P, HPC, T], BF16)   # [hd, head, t]
            kT_s = qkvp.tile([P, T], BF16)        # [hd, t]
            vN_s = qkvp.tile([P, TKC, HD], BF16)  # [tk%128, tk//128, hd]

            # ---- phases 1+2+3 interleaved per t-chunk --------------------
            # attention for chunk c only needs projections from chunks <= c,
            # so it is emitted right after chunk n=c's projections; the
            # output-projection partial for chunk c follows immediately
            # (it only needs this core's own attention output), and its
            # ReduceScatter fires while chunk c+1 computes.
            for n in range(NT):
                ts = slice(n * 512, (n + 1) * 512)
                if 1 in phases:
                    # x for this t-chunk (chunk 0's tiles were loaded up top)
                    if n == 0:
                        xhq, xlq = xhq0, xlq0
                        for pr in range(HPC // 2):
                            nc.sync.dma_start(out=wjh_s[:, pr],
                                              in_=wjh[:, pr])
                        for pr in range(HPC // 2):
                            nc.sync.dma_start(out=wjl_s[:, pr],
                                              in_=wjl[:, pr])
                    else:
                        xhq, xlq = [], []
                        for q in range(4):
                            ks = slice(q * 8 * P, (q + 1) * 8 * P)
                            xq = xsp.tile([P, 8, 512], FP8, name=f"xh{n}_{q}",
                                          tag="xh")
                            nc.gpsimd.dma_start(
                                out=xq,
                                in_=xh8[n, ks, :].rearrange("(k p) t -> p k t",
                                                            p=P))
                            xhq.append(xq)
                            xq = xsp.tile([P, 8, 512], FP8, name=f"xl{n}_{q}",
                                          tag="xl")
                            nc.gpsimd.dma_start(
                                out=xq,
                                in_=xl8[n, ks, :].rearrange("(k p) t -> p k t",
                                                            p=P))
                            xlq.append(xq)

                    def xh_sl(pr):
                        j = pr % 4
                        return xhq[pr // 4][:, 2 * j:2 * j + 2, :]

                    def xl_sl(pr):
                        j = pr % 4
                        return xlq[pr // 4][:, 2 * j:2 * j + 2, :]

                    # q-pass: one head at a time to keep PSUM pressure low;
                    # each head's rope rot-matmul is deferred into the next
                    # head's k-loop so the PE never waits on the bias add.
                    braws = {}

                    def emit_rope_q(m):
                        braw = braws.pop(m)
                        rot_ps = accp.tile([P, 512], F32, name=f"rot{n}_{m}",
                                           tag="acc")
                        nc.tensor.matmul(rot_ps, lhsT=rT_s, rhs=braw,
                                         start=True, stop=True)
                        dst = qT_s[:, m, ts]
                        tmp = stagep.tile([P, 512], F32, name=f"tmp{n}_{m}",
                                          tag="stage")
                        nc.vector.tensor_mul(tmp, rot_ps, sin_s[:, ts])
                        nc.vector.tensor_mul(dst, braw, cos_s[:, ts])
                        nc.vector.tensor_add(dst, dst, tmp)

                    for m in range(HPC):
                        pacc = accp.tile([P, 512], F32, name=f"pacc{n}_{m}",
                                         tag="acc")
                        for pr in range(NPR):     # A: xh @ wh
                            nc.tensor.matmul(
                                pacc, lhsT=woh_s[:, pr, m, :], rhs=xh_sl(pr),
                                start=(pr == 0), stop=False, perf_mode=DRSW)
                        for pr in range(NPR):     # B: xl @ wh
                            nc.tensor.matmul(
                                pacc, lhsT=woh_s[:, pr, m, :], rhs=xl_sl(pr),
                                start=False, stop=False, perf_mode=DRSW)
                        for pr in range(NPR):     # C: xh @ wl
                            nc.tensor.matmul(
                                pacc, lhsT=wol_s[:, pr, m, :], rhs=xh_sl(pr),
                                start=False, stop=(pr == NPR - 1),
                                perf_mode=DRSW)
                        braw = stagep.tile([P, 512], BF16, name=f"braw{n}_{m}",
                                           tag="braw")
                        nc.scalar.activation(
                            braw, pacc, mybir.ActivationFunctionType.Identity,
                            bias=qb_s[:, m:m + 1], scale=1.0 / S_W)
                        braws[m] = braw
                        if m > 0:
                            emit_rope_q(m - 1)
                    # kv-pass (covers the last q head's bias-add latency)
                    kacc = accp.tile([P, 512], F32, name=f"kacc{n}", tag="acc")
                    vacc = accp.tile([P, 512], F32, name=f"vacc{n}", tag="acc")
                    for wh_, wl_, acc in ((wkh_s, wkl_s, kacc),
                                          (wvh_s, wvl_s, vacc)):
                        for pr in range(NPR):
                            nc.tensor.matmul(
                                acc, lhsT=wh_[:, pr, 0, :], rhs=xh_sl(pr),
                                start=(pr == 0), stop=False, perf_mode=DRSW)
                        for pr in range(NPR):
                            nc.tensor.matmul(
                                acc, lhsT=wh_[:, pr, 0, :], rhs=xl_sl(pr),
                                start=False, stop=False, perf_mode=DRSW)
                        for pr in range(NPR):
                            nc.tensor.matmul(
                                acc, lhsT=wl_[:, pr, 0, :], rhs=xh_sl(pr),
                                start=False, stop=(pr == NPR - 1),
                                perf_mode=DRSW)
                    emit_rope_q(HPC - 1)
                    # rope for the k tile (bias add queued behind the q ones)
                    braw = stagep.tile([P, 512], BF16, name=f"brawk{n}",
                                       tag="braw")
                    nc.scalar.activation(
                        braw, kacc, mybir.ActivationFunctionType.Identity,
                        bias=kb_s[:, 0:1], scale=1.0 / S_W)
                    rot_ps = accp.tile([P, 512], F32, name=f"rotk{n}", tag="acc")
                    nc.tensor.matmul(rot_ps, lhsT=rT_s, rhs=braw,
                                     start=True, stop=True)
                    tmp = stagep.tile([P, 512], F32, name=f"tmpk{n}", tag="stage")
                    nc.vector.tensor_mul(tmp, rot_ps, sin_s[:, ts])
                    nc.vector.tensor_mul(kT_s[:, ts], braw, cos_s[:, ts])
                    nc.vector.tensor_add(kT_s[:, ts], kT_s[:, ts], tmp)

                    # v: bias then transpose into natural layout
                    v_st = stagep.tile([P, 512], BF16, name=f"vst{n}", tag="braw")
                    nc.scalar.activation(
                        v_st, vacc, mybir.ActivationFunctionType.Identity,
                        bias=vb_s[:, 0:1], scale=1.0 / S_W)
                    for j in range(4):
                        vt_ps = accp.tile([P, P], BF16, name=f"vt{n}_{j}",
                                          tag="acc")
                        nc.tensor.transpose(vt_ps, v_st[:, j * P:(j + 1) * P],
                                            ident_s)
                        nc.scalar.copy(vN_s[:, n * 4 + j, :], vt_ps)

                if 2 not in phases:
                    continue
                # ---- attention for chunk c = n ---------------------------
                c = n
                act_ks = [k for k in range(TKC) if klass[k, c] != MSK_SKIP]
                add_ks = [k for k in act_ks if klass[k, c] == MSK_ADD]
                mtiles = {}
                for k in add_ks:
                    mt = attp.tile([P, 512], BF16, name=f"mt{c}_{k}", tag="msk",
                                   bufs=max(2, len(add_ks) + 1))
                    nc.sync.dma_start(out=mt, in_=maskTb[c, k * P:(k + 1) * P, :])
                    mtiles[k] = mt
                # fp8 hi/lo of S_A*attn, heads packed as DR slot pairs
                ah_all = ooutp.tile([P, HPC, 512], FP8, name=f"ah{c}",
                                    tag="ah", bufs=2)
                al_all = ooutp.tile([P, HPC, 512], FP8, name=f"al{c}",
                                    tag="al", bufs=2)
                for h in range(HPC):
                    attn_ps = adp.tile([P, 512], F32, name=f"apv{c}_{h}",
                                       tag="attden")
                    den_ps = adp.tile([P, 512], F32, name=f"den{c}_{h}",
                                      tag="attden")
                    nact = len(act_ks)

                    # score matmuls run LOOKAHEAD tiles ahead of their
                    # exp/AV consumers so the PE stays busy through the
                    # activation-engine latency of each tile.
                    LOOKAHEAD = 2
                    s_tiles = {}

                    def emit_score(i):
                        k = act_ks[i]
                        off = 0 if i == 0 else int(col0[k, c])
                        qs = slice(c * 512 + off, (c + 1) * 512)
                        s_ps = sattp.tile([P, 512], F32, name=f"sps{c}_{h}_{k}",
                                          tag="satt")
                        nc.tensor.matmul(
                            s_ps[:, off:],
                            lhsT=kT_s[:, k * P:(k + 1) * P],
                            rhs=qT_s[:, h, qs],
                            start=True, stop=True,
                        )
                        if k in mtiles:
                            nc.vector.tensor_add(s_ps[:, off:], s_ps[:, off:],
                                                 mtiles[k][:, off:])
                        s_tiles[i] = (s_ps, off)

                    def emit_consume(i):
                        k = act_ks[i]
                        s_ps, off = s_tiles.pop(i)
                        e_sb = attp.tile([P, 512], BF16, name=f"e{c}_{h}_{k}",
                                         tag="exp", bufs=5)
                        # exp(SCALE * s + mask): mask was pre-divided by
                        # SCALE on the host, so the add can happen upstream.
                        nc.scalar.activation(
                            e_sb[:, off:], s_ps[:, off:],
                            mybir.ActivationFunctionType.Exp, scale=SCALE)
                        nc.tensor.matmul(
                            attn_ps[:, off:], lhsT=vN_s[:, k, :],
                            rhs=e_sb[:, off:],
                            start=(i == 0), stop=(i == nact - 1),
                        )
                        nc.tensor.matmul(
                            den_ps[:, off:], lhsT=ones_s, rhs=e_sb[:, off:],
                            start=(i == 0), stop=(i == nact - 1),
                        )

                    for i in range(nact):
                        emit_score(i)
                        if i >= LOOKAHEAD:
                            emit_consume(i - LOOKAHEAD)
                    for i in range(max(0, nact - LOOKAHEAD), nact):
                        emit_consume(i)

                    rcp = attp.tile([P, 512], F32, name=f"rcp{c}_{h}", tag="rcp",
                                    bufs=2)
                    nc.vector.reciprocal(rcp, den_ps)
                    attn_f = ooutp.tile([P, 512], BF16, name=f"ao{c}_{h}",
                                        tag="aout", bufs=2)
                    nc.vector.tensor_mul(attn_f, attn_ps, rcp)  # = S_A * attn
                    nc.gpsimd.tensor_copy(ah_all[:, h, :], attn_f)
                    nc.gpsimd.tensor_sub(al_all[:, h, :], attn_f,
                                         ah_all[:, h, :])

                if 3 not in phases:
                    continue
                # ---- phase 3: output-projection partial + ReduceScatter --
                # partial[out, tq] = sum_h woJ[:, h, out].T @ attn_out[h];
                # full 4096-row partial, summed across cores by the RS.
                # Groups run in pairs (kt interleaved) so the PE has work
                # while head 3's attention output finishes on the DVE, with
                # a 5-deep PSUM rotation across the acc + attden pools.
                def o_alloc(m, pool):
                    tg = "acc" if pool is accp else "attden"
                    return pool.tile([P, 512], F32, name=f"oacc{n}_{m}",
                                     tag=tg)

                def o_terms(m):
                    # 6 DR instructions: A/B/C over head-pairs 0 and 1;
                    # the pair-0 terms only need heads 0-1's fp8 outputs.
                    return [
                        (wjh_s[:, 0, m, :], ah_all[:, 0:2, :], True, False),
                        (wjh_s[:, 0, m, :], al_all[:, 0:2, :], False, False),
                        (wjl_s[:, 0, m, :], ah_all[:, 0:2, :], False, False),
                        (wjh_s[:, 1, m, :], ah_all[:, 2:4, :], False, False),
                        (wjh_s[:, 1, m, :], al_all[:, 2:4, :], False, False),
                        (wjl_s[:, 1, m, :], ah_all[:, 2:4, :], False, True),
                    ]

                def o_mm(po, term):
                    lhsT, rhs, start, stop = term
                    nc.tensor.matmul(po, lhsT=lhsT, rhs=rhs, start=start,
                                     stop=stop, perf_mode=DRSW)

                def o_finish(m, po):
                    o_sb = ooutp.tile([P, 512], BF16, name=f"o{n}_{m}",
                                      tag="osb", bufs=6)
                    # unscale (1/(S_W*S_A)) + bias; bias is nonzero only on
                    # core 0 so the RS adds it once (gpsimd cannot read PSUM,
                    # so alternate Act/DVE)
                    if m % 2 == 0:
                        nc.scalar.activation(
                            o_sb, po, mybir.ActivationFunctionType.Identity,
                            bias=ob_s[:, m:m + 1], scale=1.0 / (S_W * S_A))
                    else:
                        nc.vector.tensor_scalar(
                            o_sb, po, 1.0 / (S_W * S_A), ob_s[:, m:m + 1],
                            mybir.AluOpType.mult, mybir.AluOpType.add)
                    deng = nc.sync if m % 2 == 0 else nc.gpsimd
                    deng.dma_start(out=rs_in[n, m * P:(m + 1) * P, :],
                                   in_=o_sb)

                # groups 0-2 (accp) prefill their head-pair-0 terms while
                # heads 2-3 finish on DVE/Pool; the adp-pool groups follow.
                pos3 = [o_alloc(m, accp) for m in range(3)]
                for t in range(3):
                    for m in range(3):
                        o_mm(pos3[m], o_terms(m)[t])
                for m in range(3):
                    for t in range(3, 6):
                        o_mm(pos3[m], o_terms(m)[t])
                    o_finish(m, pos3[m])
                opools = (adp, adp, accp, accp, accp)
                for mp in range(3, NOG, 2):
                    ms = [m for m in (mp, mp + 1) if m < NOG]
                    pos = [o_alloc(m, opools[(m - 3) % 5]) for m in ms]
                    for t in range(6):
                        for i, m in enumerate(ms):
                            o_mm(pos[i], o_terms(m)[t])
                    for i, m in enumerate(ms):
                        o_finish(m, pos[i])
                nc.gpsimd.collective_compute(
                    "ReduceScatter",
                    mybir.AluOpType.add,
                    replica_groups=[list(range(NCORE))],
                    ins=[rs_in[n]],
                    outs=[rs_out[n]],
                )
                nc.gpsimd.dma_start(out=outTb[n], in_=rs_out[n])

    legalize_waits(nc)
    return nc


def _dr_pack(Wt, scale):
    """fp8 hi/lo of scale*Wt in DoubleRowSwInterleave layout.

    Wt: (K, J) lhsT (contraction-major).  Returns two [P, K//256, J//128,
    256] fp8 arrays with the two 128-ktile slots interleaved per column and
    columns reversed within each 128-block (hardware weight order).
    """
    f8 = ml_dtypes.float8_e4m3
    K, J = Wt.shape
    Ws = (np.asarray(Wt, np.float32) * np.float32(scale))
    hi = Ws.astype(f8)
    lo = (Ws - hi.astype(np.float32)).astype(f8)
    out = []
    for W in (hi, lo):
        W5 = W.reshape(K // 256, 2, P, J // P, P)[..., ::-1]
        st = np.empty((P, K // 256, J // P, 2 * P), f8)
        st[:, :, :, 0::2] = W5[:, 0].transpose(1, 0, 2, 3)
        st[:, :, :, 1::2] = W5[:, 1].transpose(1, 0, 2, 3)
        out.append(np.ascontiguousarray(st))
    return out


def _marshal_inputs(x, freqs_cos, freqs_sin, mask, wk_w, wk_b, wv_w, wv_b,
                    wo_w, wo_b):
    bf = ml_dtypes.bfloat16
    x = np.asarray(x, np.float32)
    mask = np.asarray(mask, np.float32)
    cos = np.asarray(freqs_cos, np.float32)
    sin = np.asarray(freqs_sin, np.float32)
    wk_w = np.asarray(wk_w, np.float32)
    wv_w = np.asarray(wv_w, np.float32)
    wo_w = np.asarray(wo_w, np.float32)
    wk_b = np.asarray(wk_b, np.float32)
    wv_b = np.asarray(wv_b, np.float32)
    wo_b = np.asarray(wo_b, np.float32)

    f8 = ml_dtypes.float8_e4m3
    xT = x.reshape(T, DIM).T                       # (DIM, T)
    xh = xT.astype(f8)
    xl = (xT - xh.astype(np.float32)).astype(f8)
    xh8 = np.ascontiguousarray(
        xh.reshape(DIM, NT, 512).transpose(1, 0, 2))
    xl8 = np.ascontiguousarray(
        xl.reshape(DIM, NT, 512).transpose(1, 0, 2))
    # mask applied on-device as exp(SCALE*s + SCALE*maskT): pre-divide, and
    # reblock (tq-chunk, tk, tq') so every mask tile DMA is contiguous
    maskT = mask.T / np.float32(SCALE)             # (tk, tq)
    maskTb = np.ascontiguousarray(
        maskT.reshape(T, NT, 512).transpose(1, 0, 2).astype(bf))

    cos2 = np.repeat(cos.T, 2, axis=0)  # (128, T): rows 2i,2i+1 = cos[:, i]
    sin2 = np.repeat(sin.T, 2, axis=0)

    # rotation matmul constant: out = R @ q with rot[2i] = -q[2i+1],
    # rot[2i+1] = q[2i]; lhsT layout (R transposed).
    RT = np.zeros((P, P), np.float32)
    idx = np.arange(0, P, 2)
    RT[idx + 1, idx] = -1.0
    RT[idx, idx + 1] = 1.0

    common = dict(
        xh8=xh8, xl8=xl8, maskTb=maskTb,
        cost=np.ascontiguousarray(cos2),
        sint=np.ascontiguousarray(sin2),
        rT=RT.astype(bf),
    )

    in_maps = []
    for cix in range(NCORE):
        jlo = cix * JPC
        klo = cix * HD
        m = dict(common)
        m["woh"], m["wol"] = _dr_pack(wo_w[jlo:jlo + JPC, :].T, S_W)
        m["wjh"], m["wjl"] = _dr_pack(wo_w[:, jlo:jlo + JPC].T, S_W)
        m["wkh"], m["wkl"] = _dr_pack(wk_w[klo:klo + HD, :].T, S_W)
        m["wvh"], m["wvl"] = _dr_pack(wv_w[klo:klo + HD, :].T, S_W)
        qb_ = wo_b[jlo:jlo + JPC].reshape(HPC, P).T
        kb_ = wk_b[klo:klo + HD].reshape(1, P).T
        vb_ = wv_b[klo:klo + HD].reshape(1, P).T
        # out-projection bias, added once via core 0's partial
        if cix == 0:
            ob_ = wo_b.reshape(NOG, P).T
        else:
            ob_ = np.zeros((P, NOG), np.float32)
        m["cb"] = np.ascontiguousarray(
            np.concatenate([qb_, kb_, vb_, ob_], axis=1).astype(np.float32))
        in_maps.append(m)
    return in_maps, mask


def run(inputs, trace=False):
    """Build, run on 8 cores, return (full_output, BassKernelResults)."""
    in_maps, mask = _marshal_inputs(
        inputs["x"], inputs["freqs_cos"], inputs["freqs_sin"], inputs["mask"],
        inputs["wk_w"], inputs["wk_b"], inputs["wv_w"], inputs["wv_b"],
        inputs["wo_w"], inputs["wo_b"])
    klass, col0 = _classify_mask(mask)
    nc = _build_module(klass, col0)
    res = run_bass_kernel_spmd(nc, in_maps, core_ids=list(range(NCORE)),
                               trace=trace)
    out = np.empty((DIM, T), np.float32)
    for cix in range(NCORE):
        ob = res.results[cix]["outTb"]          # (NT, JPC, 512) bf16 slices
        for n in range(NT):
            out[cix * JPC:(cix + 1) * JPC, n * 512:(n + 1) * 512] = \
                ob[n].astype(np.float32)
    out = out.T  # (T, DIM)
    return np.ascontiguousarray(out[None, :, :]).astype(np.float32), res


def kernel(**inputs):
    out, _ = run(inputs, trace=False)
    return out


# revision 35
# speedup vs baseline: 1.5647x; 1.0206x over previous
"""Tensor-parallel GQA attention prefill (B=1, T=2048, D=4096, 32 q-heads /
8 kv-heads) for 8 Trainium2 NeuronCores.

Sharding: head-parallel.  Core c owns q-heads [4c, 4c+4) and kv-head c.
  phase 1: Q/K/V projections in transposed layout (head-dim on partitions),
           RoPE applied via a rotation-matmul + two table multiplies.
  phase 2: per-head attention with scores held transposed (tk on
           partitions); softmax denominators come from a ones-matmul;
           fully-masked tiles are skipped (host inspects the mask tensor);
           score matmuls are emitted two tiles ahead of their exp/AV
           consumers so the PE never stalls on activation latency.
  phase 3: contraction-sharded output projection: each core contracts its
           own 512 attention dims (4 heads x 128, straight out of SBUF)
           against wo columns, producing a full-height [4096, 512] partial
           per t-chunk; bias is folded into core 0's partial.
  phase 3.5: per-t-chunk ReduceScatter (add) sums the partials and hands
           core r the output-dim slice [512r, 512r+512), written directly
           to the kernel output; pipelined against later chunks' compute.

Matmul operands are bf16 (fp32 accumulation in PSUM); measured end-to-end
error vs the fp32 reference is ~5e-3 relative.

NOTE: faithful to the reference "bug" -- the q projection uses wo_w/wo_b.
"""

import numpy as np
import ml_dtypes

import bass_rust
import concourse.bass as bass
import concourse.mybir as mybir
import concourse.tile as tile
from concourse.bass_utils import run_bass_kernel_spmd
from concourse.masks import make_identity

# problem constants (self-contained; do not read spec.json)
DIM = 4096
NH = 32
NKV = 8
HD = 128
T = 2048
NCORE = 8
HPC = NH // NCORE      # 4 q heads per core
JPC = HPC * HD         # 512 contraction dims owned per core
P = 128
NT = T // 512          # 4 free-dim chunks of 512
NKC = DIM // P         # 32 contraction chunks in the projections
NOG = DIM // P         # 32 output-row groups in the out projection
TKC = T // P           # 16 tk chunks in attention
SCALE = 1.0 / float(np.sqrt(HD))

F32 = mybir.dt.float32
BF16 = mybir.dt.bfloat16
FP8 = mybir.dt.float8e4
DRSW = mybir.MatmulPerfMode.DoubleRowSwInterleave

# fp8 error-compensated projections: w ~ S_W*w -> hi+lo fp8, x -> hi+lo fp8;
# x@w ~ (xh@wh + xl@wh + xh@wl)/S_W with fp32 PSUM accumulation.  DoubleRow
# processes two 128-ktile slots per instruction at 0.5 cycles/row, so the
# three terms cost 0.75x the bf16 schedule at slightly BETTER accuracy
# (the dropped xl@wl term is ~7e-4 relative).
S_W = 256.0            # weight scale (keeps fp8 out of the subnormal range)
S_A = 32.0             # attention-output scale for the out-projection rhs
NPR = NKC // 2         # 16 contraction ktile-pairs in the projections

# mask tile classification
MSK_SKIP, MSK_ZERO, MSK_ADD = 0, 1, 2


def legalize_waits(nc, max_waits=1):
    """Hoist excess on_wait conditions onto preceding nop instructions.

    This walrus build rejects instructions carrying more than a couple of
    sync-wait commands; engines execute their queue in order, so a nop that
    waits immediately before the real instruction is equivalent.
    """
    n_new = 0
    for f in nc.m.functions:
        for bb in f.blocks:
            insts = bb.instructions
            new = []
            for ins in list(insts):
                si = ins.sync_info
                waits = list(si.on_wait) if si is not None and si.on_wait else []
                if len(waits) > max_waits:
                    hoist = waits[:-max_waits]
                    keep = waits[-max_waits:]
                    for j in range(0, len(hoist), max_waits):
                        chunk = hoist[j:j + max_waits]
                        nop = mybir.InstNoOp(
                            name=f"{ins.name}_hw{j}",
                            engine=ins.engine,
                            sync_info=bass_rust.SyncInfo(
                                on_wait=chunk, on_update=[]),
                        )
                        new.append(nop)
                        n_new += 1
                    ins.sync_info = bass_rust.SyncInfo(
                        on_wait=keep,
                        on_update=list(si.on_update) if si.on_update else [])
                new.append(ins)
            insts.clear()
            insts.extend(new)
    return n_new


def _classify_mask(mask):
    """Per (tk-chunk, tq-chunk-of-512) classification of the additive mask.

    Returns (klass, col0) where col0[k, c] is the first tq column (multiple
    of 128) of the chunk that is not fully masked -- matmuls/exp for the
    columns before it are skipped (their softmax weights are exactly 0).
    """
    klass = np.empty((TKC, NT), dtype=np.int32)
    col0 = np.zeros((TKC, NT), dtype=np.int32)
    for k in range(TKC):
        for c in range(NT):
            blk = mask[c * 512:(c + 1) * 512, k * P:(k + 1) * P]
            mx = float(blk.max())
            mn = float(blk.min())
            if mx < -80.0:
                klass[k, c] = MSK_SKIP
                continue
            if mx == 0.0 and mn == 0.0:
                klass[k, c] = MSK_ZERO
            else:
                klass[k, c] = MSK_ADD
            # leading fully-masked tq columns, rounded down to 128
            colmax = blk.max(axis=1)          # per-tq-row max over this tile
            nz = np.nonzero(colmax >= -80.0)[0]
            first = int(nz[0]) if len(nz) else 0
            first = (first // P) * P
            # only safe to skip if every column before `first` is fully masked
            if first > 0 and float(blk[:first].max()) < -80.0:
                col0[k, c] = first
    # never allow a fully-empty (all-skip) tq chunk; keep one tile live
    for c in range(NT):
        if all(klass[k, c] == MSK_SKIP for k in range(TKC)):
            klass[min(c * 4, TKC - 1), c] = MSK_ADD
    return klass, col0


def _build_module(klass, col0, phases=(1, 2, 3)):
    nc = bass.Bass()

    # inputs are pre-reblocked on the host so every DMA is contiguous;
    # weights arrive as fp8 hi/lo pairs in DoubleRowSwInterleave layout
    # ([p, ktile-pair, m-block, 256] with A/B slots interleaved per column
    # and columns reversed within each 128-block).
    xh8 = nc.declare_dram_parameter("xh8", [NT, DIM, 512], FP8, isOutput=False)
    xl8 = nc.declare_dram_parameter("xl8", [NT, DIM, 512], FP8, isOutput=False)
    woh = nc.declare_dram_parameter("woh", [P, NPR, HPC, 2 * P], FP8,
                                    isOutput=False)
    wol = nc.declare_dram_parameter("wol", [P, NPR, HPC, 2 * P], FP8,
                                    isOutput=False)
    wkh = nc.declare_dram_parameter("wkh", [P, NPR, 1, 2 * P], FP8,
                                    isOutput=False)
    wkl = nc.declare_dram_parameter("wkl", [P, NPR, 1, 2 * P], FP8,
                                    isOutput=False)
    wvh = nc.declare_dram_parameter("wvh", [P, NPR, 1, 2 * P], FP8,
                                    isOutput=False)
    wvl = nc.declare_dram_parameter("wvl", [P, NPR, 1, 2 * P], FP8,
                                    isOutput=False)
    wjh = nc.declare_dram_parameter("wjh", [P, HPC // 2, NOG, 2 * P], FP8,
                                    isOutput=False)
    wjl = nc.declare_dram_parameter("wjl", [P, HPC // 2, NOG, 2 * P], FP8,
                                    isOutput=False)
    # packed biases: [qb(HPC) | kb(1) | vb(1) | ob(NOG)] -- one DMA
    cb = nc.declare_dram_parameter("cb", [P, HPC + 2 + NOG], F32,
                                   isOutput=False)
    maskTb = nc.declare_dram_parameter("maskTb", [NT, T, 512], BF16,
                                       isOutput=False)
    cost = nc.declare_dram_parameter("cost", [P, T], F32, isOutput=False)
    sint = nc.declare_dram_parameter("sint", [P, T], F32, isOutput=False)
    rT = nc.declare_dram_parameter("rT", [P, P], BF16, isOutput=False)
    outTb = nc.declare_dram_parameter("outTb", [NT, JPC, 512], BF16,
                                      isOutput=True)

    rs_in = nc.dram_tensor("rs_in", [NT, DIM, 512], BF16)
    rs_out = nc.dram_tensor("rs_out", [NT, JPC, 512], BF16)

    with tile.TileContext(nc) as tc:
        with (
            tc.tile_pool(name="wpool", bufs=1) as wpool,
            tc.tile_pool(name="const", bufs=1) as constp,
            tc.tile_pool(name="qkv", bufs=1) as qkvp,
            tc.tile_pool(name="xs", bufs=5) as xsp,
            tc.tile_pool(name="stage", bufs=3) as stagep,
            tc.tile_pool(name="att", bufs=4) as attp,
            tc.tile_pool(name="oout", bufs=6) as ooutp,
            tc.tile_pool(name="acc", bufs=3, space="PSUM") as accp,
            tc.tile_pool(name="satt", bufs=3, space="PSUM") as sattp,
            tc.tile_pool(name="attden", bufs=2, space="PSUM") as adp,
        ):
            # ---- chunk-0 x-hi tiles + wo-hi pieces, interleaved: the first
            # k-loop's A-terms consume both streams in ktile order, so
            # neither may monopolize the DMA device; the lo streams follow
            # (their B/C terms run after each head's A accumulation) -------
            xhq0 = [xsp.tile([P, 8, 512], FP8, name=f"xh0_{q}", tag="xh")
                    for q in range(4)]
            xlq0 = [xsp.tile([P, 8, 512], FP8, name=f"xl0_{q}", tag="xl")
                    for q in range(4)]
            woh_s = wpool.tile([P, NPR, HPC, 2 * P], FP8)
            wol_s = wpool.tile([P, NPR, HPC, 2 * P], FP8)
            x_parts = []
            for q in range(4):
                lim = (0, 2, 8) if q == 0 else (0, 4, 8)
                for ha, hb in zip(lim[:-1], lim[1:]):
                    x_parts.append((q, ha, hb))
            woh_parts = [(0, 1), (1, 2), (2, 3), (3, 4), (4, 6), (6, 8),
                         (8, 10), (10, 12), (12, 14), (14, 16)]
            for i in range(max(len(x_parts), len(woh_parts))):
                if i < len(x_parts):
                    q, ha, hb = x_parts[i]
                    ks = slice(q * 8 * P, (q + 1) * 8 * P)
                    nc.gpsimd.dma_start(
                        out=xhq0[q][:, ha:hb, :],
                        in_=xh8[0, ks][ha * P:hb * P, :].rearrange(
                            "(k p) t -> p k t", p=P))
                if i < len(woh_parts):
                    plo, phi = woh_parts[i]
                    nc.sync.dma_start(out=woh_s[:, plo:phi],
                                      in_=woh[:, plo:phi])
            # lo streams for chunk 0 (B-terms start ~2us into head 0)
            for q in range(4):
                ks = slice(q * 8 * P, (q + 1) * 8 * P)
                nc.gpsimd.dma_start(
                    out=xlq0[q],
                    in_=xl8[0, ks, :].rearrange("(k p) t -> p k t", p=P))
                nc.sync.dma_start(out=wol_s[:, 4 * q:4 * q + 4],
                                  in_=wol[:, 4 * q:4 * q + 4])

            wkh_s = wpool.tile([P, NPR, 1, 2 * P], FP8)
            wvh_s = wpool.tile([P, NPR, 1, 2 * P], FP8)
            wkl_s = wpool.tile([P, NPR, 1, 2 * P], FP8)
            wvl_s = wpool.tile([P, NPR, 1, 2 * P], FP8)
            nc.sync.dma_start(out=wkh_s, in_=wkh[:, :, :, :])
            nc.sync.dma_start(out=wvh_s, in_=wvh[:, :, :, :])
            nc.sync.dma_start(out=wkl_s, in_=wkl[:, :, :, :])
            nc.sync.dma_start(out=wvl_s, in_=wvl[:, :, :, :])
            # out-projection weights, loaded piecewise after chunk 0's x
            # (first needed by the chunk-0 output projection, much later)
            wjh_s = wpool.tile([P, HPC // 2, NOG, 2 * P], FP8)
            wjl_s = wpool.tile([P, HPC // 2, NOG, 2 * P], FP8)

            cos_s = constp.tile([P, T], F32)
            sin_s = constp.tile([P, T], F32)
            nc.sync.dma_start(out=cos_s, in_=cost[:, :])
            nc.sync.dma_start(out=sin_s, in_=sint[:, :])

            rT_s = constp.tile([P, P], BF16)
            nc.sync.dma_start(out=rT_s, in_=rT[:, :])
            cb_s = constp.tile([P, HPC + 2 + NOG], F32)
            nc.sync.dma_start(out=cb_s, in_=cb[:, :])
            qb_s = cb_s[:, 0:HPC]
            kb_s = cb_s[:, HPC:HPC + 1]
            vb_s = cb_s[:, HPC + 1:HPC + 2]
            ob_s = cb_s[:, HPC + 2:]

            # den accumulates sum(e)/S_A so attn_f = attn_ps * rcp comes out
            # pre-scaled by S_A for the fp8 split (1/32 is exact in bf16)
            ones_s = constp.tile([P, P], BF16)
            nc.vector.memset(ones_s, 1.0 / S_A)
            ident_s = constp.tile([P, P], BF16)
            make_identity(nc, ident_s)

            # persistent Q/K/V in rope-d transposed layout
            qT_s = qkvp.tile([P, HPC, T], BF16)   # [hd, head, t]
            kT_s = qkvp.tile([P, T], BF16)        # [hd, t]
            vN_s = qkvp.tile([P, TKC, HD], BF16)  # [tk%128, tk//128, hd]

            # ---- phases 1+2+3 interleaved per t-chunk --------------------
            # attention for chunk c only needs projections from chunks <= c,
            # so it is emitted right after chunk n=c's projections; the
            # output-projection partial for chunk c follows immediately
            # (it only needs this core's own attention output), and its
            # ReduceScatter fires while chunk c+1 computes.
            for n in range(NT):
                ts = slice(n * 512, (n + 1) * 512)
                if 1 in phases:
                    # x for this t-chunk (chunk 0's tiles were loaded up top)
                    if n == 0:
                        xhq, xlq = xhq0, xlq0
                        for pr in range(HPC // 2):
                            nc.sync.dma_start(out=wjh_s[:, pr],
                                              in_=wjh[:, pr])
                        for pr in range(HPC // 2):
                            nc.sync.dma_start(out=wjl_s[:, pr],
                                              in_=wjl[:, pr])
                    else:
                        xhq, xlq = [], []
                        for q in range(4):
                            ks = slice(q * 8 * P, (q + 1) * 8 * P)
                            xq = xsp.tile([P, 8, 512], FP8, name=f"xh{n}_{q}",
                                          tag="xh")
                            nc.gpsimd.dma_start(
                                out=xq,
                                in_=xh8[n, ks, :].rearrange("(k p) t -> p k t",
                                                            p=P))
                            xhq.append(xq)
                            xq = xsp.tile([P, 8, 512], FP8, name=f"xl{n}_{q}",
                                          tag="xl")
                            nc.gpsimd.dma_start(
                                out=xq,
                                in_=xl8[n, ks, :].rearrange("(k p) t -> p k t",
                                                            p=P))
                            xlq.append(xq)

                    def xh_sl(pr):
                        j = pr % 4
                        return xhq[pr // 4][:, 2 * j:2 * j + 2, :]

                    def xl_sl(pr):
                        j = pr % 4
                        return xlq[pr // 4][:, 2 * j:2 * j + 2, :]

                    # q-pass: one head at a time to keep PSUM pressure low;
                    # each head's rope rot-matmul is deferred into the next
                    # head's k-loop so the PE never waits on the bias add.
                    braws = {}

                    def emit_rope_q(m):
                        braw = braws.pop(m)
                        rot_ps = accp.tile([P, 512], F32, name=f"rot{n}_{m}",
                                           tag="acc")
                        nc.tensor.matmul(rot_ps, lhsT=rT_s, rhs=braw,
                                         start=True, stop=True)
                        dst = qT_s[:, m, ts]
                        tmp = stagep.tile([P, 512], F32, name=f"tmp{n}_{m}",
                                          tag="stage")
                        nc.vector.tensor_mul(tmp, rot_ps, sin_s[:, ts])
                        nc.vector.tensor_mul(dst, braw, cos_s[:, ts])
                        nc.vector.tensor_add(dst, dst, tmp)

                    # A-terms lead their head's B/C terms by one head, so
                    # the very first k-loop only needs the hi DMA streams
                    # (xh/woh) -- the lo streams arrive during A(1).
                    paccs = {}

                    def emit_A(m):
                        pacc = accp.tile([P, 512], F32, name=f"pacc{n}_{m}",
                                         tag="acc")
                        for pr in range(NPR):     # A: xh @ wh
                            nc.tensor.matmul(
                                pacc, lhsT=woh_s[:, pr, m, :], rhs=xh_sl(pr),
                                start=(pr == 0), stop=False, perf_mode=DRSW)
                        paccs[m] = pacc

                    def emit_BC(m):
                        pacc = paccs.pop(m)
                        for pr in range(NPR):     # B: xl @ wh
                            nc.tensor.matmul(
                                pacc, lhsT=woh_s[:, pr, m, :], rhs=xl_sl(pr),
                                start=False, stop=False, perf_mode=DRSW)
                        for pr in range(NPR):     # C: xh @ wl
                            nc.tensor.matmul(
                                pacc, lhsT=wol_s[:, pr, m, :], rhs=xh_sl(pr),
                                start=False, stop=(pr == NPR - 1),
                                perf_mode=DRSW)
                        braw = stagep.tile([P, 512], BF16, name=f"braw{n}_{m}",
                                           tag="braw")
                        nc.scalar.activation(
                            braw, pacc, mybir.ActivationFunctionType.Identity,
                            bias=qb_s[:, m:m + 1], scale=1.0 / S_W)
                        braws[m] = braw

                    for m in range(HPC):
                        emit_A(m)
                        if m > 0:
                            emit_BC(m - 1)
                        if m > 1:
                            emit_rope_q(m - 2)
                    emit_BC(HPC - 1)
                    emit_rope_q(HPC - 2)
                    # kv-pass (covers the last q head's bias-add latency)
                    kacc = accp.tile([P, 512], F32, name=f"kacc{n}", tag="acc")
                    vacc = accp.tile([P, 512], F32, name=f"vacc{n}", tag="acc")
                    for wh_, wl_, acc in ((wkh_s, wkl_s, kacc),
                                          (wvh_s, wvl_s, vacc)):
                        for pr in range(NPR):
                            nc.tensor.matmul(
                                acc, lhsT=wh_[:, pr, 0, :], rhs=xh_sl(pr),
                                start=(pr == 0), stop=False, perf_mode=DRSW)
                        for pr in range(NPR):
                            nc.tensor.matmul(
                                acc, lhsT=wh_[:, pr, 0, :], rhs=xl_sl(pr),
                                start=False, stop=False, perf_mode=DRSW)
                        for pr in range(NPR):
                            nc.tensor.matmul(
                                acc, lhsT=wl_[:, pr, 0, :], rhs=xh_sl(pr),
                                start=False, stop=(pr == NPR - 1),
                                perf_mode=DRSW)
                    emit_rope_q(HPC - 1)
                    # rope for the k tile (bias add queued behind the q ones)
                    braw = stagep.tile([P, 512], BF16, name=f"brawk{n}",
                                       tag="braw")
                    nc.scalar.activation(
                        braw, kacc, mybir.ActivationFunctionType.Identity,
                        bias=kb_s[:, 0:1], scale=1.0 / S_W)
                    rot_ps = accp.tile([P, 512], F32, name=f"rotk{n}", tag="acc")
                    nc.tensor.matmul(rot_ps, lhsT=rT_s, rhs=braw,
                                     start=True, stop=True)
                    tmp = stagep.tile([P, 512], F32, name=f"tmpk{n}", tag="stage")
                    nc.vector.tensor_mul(tmp, rot_ps, sin_s[:, ts])
                    nc.vector.tensor_mul(kT_s[:, ts], braw, cos_s[:, ts])
                    nc.vector.tensor_add(kT_s[:, ts], kT_s[:, ts], tmp)

                    # v: bias then transpose into natural layout
                    v_st = stagep.tile([P, 512], BF16, name=f"vst{n}", tag="braw")
                    nc.scalar.activation(
                        v_st, vacc, mybir.ActivationFunctionType.Identity,
                        bias=vb_s[:, 0:1], scale=1.0 / S_W)
                    for j in range(4):
                        vt_ps = accp.tile([P, P], BF16, name=f"vt{n}_{j}",
                                          tag="acc")
                        nc.tensor.transpose(vt_ps, v_st[:, j * P:(j + 1) * P],
                                            ident_s)
                        nc.scalar.copy(vN_s[:, n * 4 + j, :], vt_ps)

                if 2 not in phases:
                    continue
                # ---- attention for chunk c = n ---------------------------
                c = n
                act_ks = [k for k in range(TKC) if klass[k, c] != MSK_SKIP]
                add_ks = [k for k in act_ks if klass[k, c] == MSK_ADD]
                mtiles = {}
                for k in add_ks:
                    mt = attp.tile([P, 512], BF16, name=f"mt{c}_{k}", tag="msk",
                                   bufs=max(2, len(add_ks) + 1))
                    nc.sync.dma_start(out=mt, in_=maskTb[c, k * P:(k + 1) * P, :])
                    mtiles[k] = mt
                # fp8 hi/lo of S_A*attn, heads packed as DR slot pairs
                ah_all = ooutp.tile([P, HPC, 512], FP8, name=f"ah{c}",
                                    tag="ah", bufs=2)
                al_all = ooutp.tile([P, HPC, 512], FP8, name=f"al{c}",
                                    tag="al", bufs=2)
                # score matmuls run LOOKAHEAD tiles ahead of their exp/AV
                # consumers -- ACROSS head boundaries -- so the PE stays
                # busy through the activation-engine latency of each tile.
                # apv/den PSUM pairs alternate between the adp and acc
                # pools so the next head never waits on the previous one.
                nact = len(act_ks)
                LOOKAHEAD = 2
                s_tiles = {}
                hps = {}

                def h_pair(h):
                    if h not in hps:
                        pool = (adp, accp)[h % 2]
                        tg = "attden" if pool is adp else "acc"
                        hps[h] = (
                            pool.tile([P, 512], F32, name=f"apv{c}_{h}",
                                      tag=tg),
                            pool.tile([P, 512], F32, name=f"den{c}_{h}",
                                      tag=tg),
                        )
                    return hps[h]

                def emit_score(h, i):
                    k = act_ks[i]
                    off = 0 if i == 0 else int(col0[k, c])
                    qs = slice(c * 512 + off, (c + 1) * 512)
                    s_ps = sattp.tile([P, 512], F32, name=f"sps{c}_{h}_{k}",
                                      tag="satt")
                    nc.tensor.matmul(
                        s_ps[:, off:],
                        lhsT=kT_s[:, k * P:(k + 1) * P],
                        rhs=qT_s[:, h, qs],
                        start=True, stop=True,
                    )
                    if k in mtiles:
                        nc.vector.tensor_add(s_ps[:, off:], s_ps[:, off:],
                                             mtiles[k][:, off:])
                    s_tiles[(h, i)] = (s_ps, off)

                def emit_consume(h, i):
                    k = act_ks[i]
                    s_ps, off = s_tiles.pop((h, i))
                    attn_ps, den_ps = h_pair(h)
                    e_sb = attp.tile([P, 512], BF16, name=f"e{c}_{h}_{k}",
                                     tag="exp", bufs=5)
                    # exp(SCALE * s + mask): mask was pre-divided by
                    # SCALE on the host, so the add can happen upstream.
                    nc.scalar.activation(
                        e_sb[:, off:], s_ps[:, off:],
                        mybir.ActivationFunctionType.Exp, scale=SCALE)
                    nc.tensor.matmul(
                        attn_ps[:, off:], lhsT=vN_s[:, k, :],
                        rhs=e_sb[:, off:],
                        start=(i == 0), stop=(i == nact - 1),
                    )
                    nc.tensor.matmul(
                        den_ps[:, off:], lhsT=ones_s, rhs=e_sb[:, off:],
                        start=(i == 0), stop=(i == nact - 1),
                    )
                    if i == nact - 1:
                        finish_head(h)

                def finish_head(h):
                    attn_ps, den_ps = hps.pop(h)
                    rcp = attp.tile([P, 512], F32, name=f"rcp{c}_{h}",
                                    tag="rcp", bufs=2)
                    nc.vector.reciprocal(rcp, den_ps)
                    attn_f = ooutp.tile([P, 512], BF16, name=f"ao{c}_{h}",
                                        tag="aout", bufs=2)
                    nc.vector.tensor_mul(attn_f, attn_ps, rcp)  # = S_A*attn
                    nc.gpsimd.tensor_copy(ah_all[:, h, :], attn_f)
                    nc.gpsimd.tensor_sub(al_all[:, h, :], attn_f,
                                         ah_all[:, h, :])

                flat = [(h, i) for h in range(HPC) for i in range(nact)]
                for j, (h, i) in enumerate(flat):
                    emit_score(h, i)
                    if j >= LOOKAHEAD:
                        emit_consume(*flat[j - LOOKAHEAD])
                for j in range(max(0, len(flat) - LOOKAHEAD), len(flat)):
                    emit_consume(*flat[j])

                if 3 not in phases:
                    continue
                # ---- phase 3: output-projection partial + ReduceScatter --
                # partial[out, tq] = sum_h woJ[:, h, out].T @ attn_out[h];
                # full 4096-row partial, summed across cores by the RS.
                # Groups run in pairs (kt interleaved) so the PE has work
                # while head 3's attention output finishes on the DVE, with
                # a 5-deep PSUM rotation across the acc + attden pools.
                def o_alloc(m, pool):
                    tg = "acc" if pool is accp else "attden"
                    return pool.tile([P, 512], F32, name=f"oacc{n}_{m}",
                                     tag=tg)

                def o_terms(m):
                    # 6 DR instructions: A/B/C over head-pairs 0 and 1;
                    # the pair-0 terms only need heads 0-1's fp8 outputs.
                    return [
                        (wjh_s[:, 0, m, :], ah_all[:, 0:2, :], True, False),
                        (wjh_s[:, 0, m, :], al_all[:, 0:2, :], False, False),
                        (wjl_s[:, 0, m, :], ah_all[:, 0:2, :], False, False),
                        (wjh_s[:, 1, m, :], ah_all[:, 2:4, :], False, False),
                        (wjh_s[:, 1, m, :], al_all[:, 2:4, :], False, False),
                        (wjl_s[:, 1, m, :], ah_all[:, 2:4, :], False, True),
                    ]

                def o_mm(po, term):
                    lhsT, rhs, start, stop = term
                    nc.tensor.matmul(po, lhsT=lhsT, rhs=rhs, start=start,
                                     stop=stop, perf_mode=DRSW)

                def o_finish(m, po):
                    o_sb = ooutp.tile([P, 512], BF16, name=f"o{n}_{m}",
                                      tag="osb", bufs=6)
                    # unscale (1/(S_W*S_A)) + bias; bias is nonzero only on
                    # core 0 so the RS adds it once (gpsimd cannot read PSUM,
                    # so alternate Act/DVE)
                    if m % 2 == 0:
                        nc.scalar.activation(
                            o_sb, po, mybir.ActivationFunctionType.Identity,
                            bias=ob_s[:, m:m + 1], scale=1.0 / (S_W * S_A))
                    else:
                        nc.vector.tensor_scalar(
                            o_sb, po, 1.0 / (S_W * S_A), ob_s[:, m:m + 1],
                            mybir.AluOpType.mult, mybir.AluOpType.add)
                    deng = nc.sync if m % 2 == 0 else nc.gpsimd
                    deng.dma_start(out=rs_in[n, m * P:(m + 1) * P, :],
                                   in_=o_sb)

                # groups 0-2 prefill their head-pair-0 terms while heads
                # 2-3 finish on DVE/Pool; their PSUM buffers come from the
                # pools attention released earliest (h0/h2 used adp).
                opools = (adp, adp, accp, accp, accp)
                pos3 = [o_alloc(m, opools[m % 5]) for m in range(3)]
                for t in range(3):
                    for m in range(3):
                        o_mm(pos3[m], o_terms(m)[t])
                for m in range(3):
                    for t in range(3, 6):
                        o_mm(pos3[m], o_terms(m)[t])
                    o_finish(m, pos3[m])
                for mp in range(3, NOG, 2):
                    ms = [m for m in (mp, mp + 1) if m < NOG]
                    pos = [o_alloc(m, opools[m % 5]) for m in ms]
                    for t in range(6):
                        for i, m in enumerate(ms):
                            o_mm(pos[i], o_terms(m)[t])
                    for i, m in enumerate(ms):
                        o_finish(m, pos[i])
                nc.gpsimd.collective_compute(
                    "ReduceScatter",
                    mybir.AluOpType.add,
                    replica_groups=[list(range(NCORE))],
                    ins=[rs_in[n]],
                    outs=[rs_out[n]],
                )
                nc.gpsimd.dma_start(out=outTb[n], in_=rs_out[n])

    legalize_waits(nc)
    return nc


def _dr_pack(Wt, scale):
    """fp8 hi/lo of scale*Wt in DoubleRowSwInterleave layout.

    Wt: (K, J) lhsT (contraction-major).  Returns two [P, K//256, J//128,
    256] fp8 arrays with the two 128-ktile slots interleaved per column and
    columns reversed within each 128-block (hardware weight order).
    """
    f8 = ml_dtypes.float8_e4m3
    K, J = Wt.shape
    Ws = (np.asarray(Wt, np.float32) * np.float32(scale))
    hi = Ws.astype(f8)
    lo = (Ws - hi.astype(np.float32)).astype(f8)
    out = []
    for W in (hi, lo):
        W5 = W.reshape(K // 256, 2, P, J // P, P)[..., ::-1]
        st = np.empty((P, K // 256, J // P, 2 * P), f8)
        st[:, :, :, 0::2] = W5[:, 0].transpose(1, 0, 2, 3)
        st[:, :, :, 1::2] = W5[:, 1].transpose(1, 0, 2, 3)
        out.append(np.ascontiguousarray(st))
    return out


def _marshal_inputs(x, freqs_cos, freqs_sin, mask, wk_w, wk_b, wv_w, wv_b,
                    wo_w, wo_b):
    bf = ml_dtypes.bfloat16
    x = np.asarray(x, np.float32)
    mask = np.asarray(mask, np.float32)
    cos = np.asarray(freqs_cos, np.float32)
    sin = np.asarray(freqs_sin, np.float32)
    wk_w = np.asarray(wk_w, np.float32)
    wv_w = np.asarray(wv_w, np.float32)
    wo_w = np.asarray(wo_w, np.float32)
    wk_b = np.asarray(wk_b, np.float32)
    wv_b = np.asarray(wv_b, np.float32)
    wo_b = np.asarray(wo_b, np.float32)

    f8 = ml_dtypes.float8_e4m3
    xT = x.reshape(T, DIM).T                       # (DIM, T)
    xh = xT.astype(f8)
    xl = (xT - xh.astype(np.float32)).astype(f8)
    xh8 = np.ascontiguousarray(
        xh.reshape(DIM, NT, 512).transpose(1, 0, 2))
    xl8 = np.ascontiguousarray(
        xl.reshape(DIM, NT, 512).transpose(1, 0, 2))
    # mask applied on-device as exp(SCALE*s + SCALE*maskT): pre-divide, and
    # reblock (tq-chunk, tk, tq') so every mask tile DMA is contiguous
    maskT = mask.T / np.float32(SCALE)             # (tk, tq)
    maskTb = np.ascontiguousarray(
        maskT.reshape(T, NT, 512).transpose(1, 0, 2).astype(bf))

    cos2 = np.repeat(cos.T, 2, axis=0)  # (128, T): rows 2i,2i+1 = cos[:, i]
    sin2 = np.repeat(sin.T, 2, axis=0)

    # rotation matmul constant: out = R @ q with rot[2i] = -q[2i+1],
    # rot[2i+1] = q[2i]; lhsT layout (R transposed).
    RT = np.zeros((P, P), np.float32)
    idx = np.arange(0, P, 2)
    RT[idx + 1, idx] = -1.0
    RT[idx, idx + 1] = 1.0

    common = dict(
        xh8=xh8, xl8=xl8, maskTb=maskTb,
        cost=np.ascontiguousarray(cos2),
        sint=np.ascontiguousarray(sin2),
        rT=RT.astype(bf),
    )

    in_maps = []
    for cix in range(NCORE):
        jlo = cix * JPC
        klo = cix * HD
        m = dict(common)
        m["woh"], m["wol"] = _dr_pack(wo_w[jlo:jlo + JPC, :].T, S_W)
        m["wjh"], m["wjl"] = _dr_pack(wo_w[:, jlo:jlo + JPC].T, S_W)
        m["wkh"], m["wkl"] = _dr_pack(wk_w[klo:klo + HD, :].T, S_W)
        m["wvh"], m["wvl"] = _dr_pack(wv_w[klo:klo + HD, :].T, S_W)
        qb_ = wo_b[jlo:jlo + JPC].reshape(HPC, P).T
        kb_ = wk_b[klo:klo + HD].reshape(1, P).T
        vb_ = wv_b[klo:klo + HD].reshape(1, P).T
        # out-projection bias, added once via core 0's partial
        if cix == 0:
            ob_ = wo_b.reshape(NOG, P).T
        else:
            ob_ = np.zeros((P, NOG), np.float32)
        m["cb"] = np.ascontiguousarray(
            np.concatenate([qb_, kb_, vb_, ob_], axis=1).astype(np.float32))
        in_maps.append(m)
    return in_maps, mask


def run(inputs, trace=False):
    """Build, run on 8 cores, return (full_output, BassKernelResults)."""
    in_maps, mask = _marshal_inputs(
        inputs["x"], inputs["freqs_cos"], inputs["freqs_sin"], inputs["mask"],
        inputs["wk_w"], inputs["wk_b"], inputs["wv_w"], inputs["wv_b"],
        inputs["wo_w"], inputs["wo_b"])
    klass, col0 = _classify_mask(mask)
    nc = _build_module(klass, col0)
    res = run_bass_kernel_spmd(nc, in_maps, core_ids=list(range(NCORE)),
                               trace=trace)
    out = np.empty((DIM, T), np.float32)
    for cix in range(NCORE):
        ob = res.results[cix]["outTb"]          # (NT, JPC, 512) bf16 slices
        for n in range(NT):
            out[cix * JPC:(cix + 1) * JPC, n * 512:(n + 1) * 512] = \
                ob[n].astype(np.float32)
    out = out.T  # (T, DIM)
    return np.ascontiguousarray(out[None, :, :]).astype(np.float32), res


def kernel(**inputs):
    out, _ = run(inputs, trace=False)
    return out
